# revision 1
# baseline (speedup 1.0000x reference)
"""Trainium2 Bass kernel for nn_Classifier_22625887715977 (sparse_attention).

kernel(**inputs) takes FULL unsharded inputs (bs=32) and returns the full
[32, 75, 6] logits. Shards the batch over 8 NeuronCores (4 episodes per
core); weights replicated and streamed.

Math (per episode, exact reassociation of the reference — never materializes
the expanded per-(episode,way) base bank):
  s      = leaky(ss @ Wm1 + bm1) @ Wm2 + bm2
  avg    = mean_n [bw | bsm]
  gvis   = sigmoid(avg @ Wvis + bvis) + 1 ; gsem = sigmoid(avg @ Wsem + bsem) + 1
  q      = sc @ Wq + s @ Wqs
  scores = ((q @ Wk^T) * gvis) @ bw^T + ((q @ Wks^T) * gsem) @ bsm^T ; attn = softmax(scores/32)
  out    = ((attn @ bw) * gvis) @ Wv ; out2 = out @ Wfc + sc
  fake   = mean_w out2 ; protos = [sc; fake] ; logits = temp * cos(qf, protos)

Implementation notes:
 - fp32r (full-rate fp32 matmul mode, ~1e-3 rel err) on the wide-N matmuls;
   exact fp32 on PE transposes and the final cosine path.
 - Transposed ("feature-on-partitions") layouts so episode packing happens on
   free dims (partition bases stay 32-aligned).
 - Three DMA queues: small loads on gpsimd (SWDGE), banks on sync (HWDGE/SP),
   weight streams on scalar (HWDGE/ACT), emitted at use sites.
 - PSUM accumulators are single-bank [*,512] halves.
"""

import numpy as np

BS = 32
NCORES = 8
EPC = BS // NCORES
NW = 5
B20 = EPC * NW
FD = 1024
FDC = FD // 128
SEM = 300
SEMCH = [(0, 128), (128, 128), (256, 44)]
NB = 512
NBC = NB // 128
NQ = 75
NPROTO = NW + 1

_MODULE_CACHE = {}


def _build_module(temp: float):
    import concourse.bass as bass
    import concourse.mybir as mybir
    import concourse.tile as tile
    from concourse import bacc

    f32 = mybir.dt.float32
    f32r = mybir.dt.float32r
    AF = mybir.ActivationFunctionType
    ALU = mybir.AluOpType
    AX = mybir.AxisListType

    nc = bacc.Bacc("TRN2", target_bir_lowering=False, debug=False)

    di = lambda name, shape: nc.dram_tensor(name, shape, f32, kind="ExternalInput")
    sc_d = di("support_center", [EPC, NW, FD])
    bw_d = di("base_weights", [EPC, NB, FD])
    ss_d = di("support_seman", [EPC, NW, SEM])
    bsm_d = di("base_seman", [EPC, NB, SEM])
    qf_d = di("query_feature", [EPC, NQ, FD])
    wm1_d = di("Wm1", [SEM, SEM])
    bm1_d = di("bm1", [SEM, 1])
    wm2_d = di("Wm2", [SEM, SEM])
    bm2_d = di("bm2", [SEM, 1])
    wvis_d = di("Wvis", [FD + SEM, FD])
    bvis_d = di("bvis", [1, FD])
    wsem_d = di("Wsem", [FD + SEM, SEM])
    bsem_d = di("bsem", [1, SEM])
    wq_d = di("Wq", [FD, FD])
    wk_d = di("Wk", [FD, FD])
    wv_d = di("Wv", [FD, FD])
    wqs_d = di("Wqs", [SEM, FD])
    wks_d = di("Wks", [SEM, FD])
    wfc_d = di("Wfc", [FD, FD])
    ident_d = di("aux_ident", [128, 128])
    inv512_d = di("aux_inv512", [128, 1])
    one4_d = di("aux_one4", [1, EPC])
    fifths_d = di("aux_fifths", [B20, EPC])
    out_d = nc.dram_tensor("out", [EPC, NQ, NPROTO], f32, kind="ExternalOutput")

    from contextlib import ExitStack
    with tile.TileContext(nc) as tc, ExitStack() as _ctx:
        def _pool(**kw):
            return _ctx.enter_context(tc.tile_pool(**kw))
        cpool = _pool(name="const", bufs=1)
        wres = _pool(name="wres", bufs=1)
        wbig = _pool(name="wbig", bufs=2)
        wgate = _pool(name="wgate", bufs=2)
        wktp = _pool(name="wkt", bufs=2)
        wlt = _pool(name="wlate", bufs=2)
        wsm = _pool(name="wsem", bufs=2)
        wkc = _pool(name="wkcol", bufs=2)
        bpool = _pool(name="banks", bufs=EPC)
        apool = _pool(name="acts", bufs=1)
        npool = _pool(name="nat4k", bufs=2)
        npool_s = _pool(name="nat12", bufs=2)
        sqpool = _pool(name="sq4k", bufs=1)
        qpool = _pool(name="qfp", bufs=2)
        qntp = _pool(name="qnt", bufs=4)
        smp = _pool(name="smalls", bufs=1)
        spool2 = _pool(name="stage2", bufs=2)
        pt = _pool(name="pt", bufs=3, space="PSUM")
        pacc = _pool(name="pacc", bufs=3, space="PSUM")
        psm = _pool(name="ps1", bufs=2, space="PSUM")
        if True:
            # ================= early small loads on sync, then banks =================
            ident = cpool.tile([128, 128], f32, tag="ident")
            nc.sync.dma_start(ident[:], ident_d.ap())
            identr = cpool.tile([128, 128], f32r, tag="identr")
            nc.sync.dma_start(identr[:], ident_d.ap().bitcast(f32r))
            sc_nat = apool.tile([B20, FD], f32, tag="sc_nat")
            nc.sync.dma_start(sc_nat[:], sc_d.ap().rearrange("e w d -> (e w) d"))
            ss_nat = apool.tile([B20, SEM], f32, tag="ss_nat")
            nc.sync.dma_start(ss_nat[:], ss_d.ap().rearrange("e w d -> (e w) d"))

            # ================= banks on the scalar queue =================
            bw_nat, bsm_nat = [], []
            for e in range(EPC):
                bwt = bpool.tile([128, NBC, FD], f32r, tag="bw")
                nc.scalar.dma_start(bwt[:], bw_d.ap()[e].rearrange("(c p) d -> p c d", p=128).bitcast(f32r))
                bw_nat.append(bwt)
                bst = bpool.tile([128, NBC, SEM], f32r, tag="bsm")
                nc.scalar.dma_start(bst[:], bsm_d.ap()[e].rearrange("(c p) d -> p c d", p=128).bitcast(f32r))
                bsm_nat.append(bst)

            # ================= small loads on the gpsimd (SWDGE) queue =======
            inv512 = cpool.tile([128, 1], f32r, tag="inv512")
            nc.gpsimd.dma_start(inv512[:], inv512_d.ap().bitcast(f32r))
            one4 = cpool.tile([1, EPC], f32r, tag="one4")
            nc.gpsimd.dma_start(one4[:], one4_d.ap().bitcast(f32r))
            fifths = cpool.tile([B20, EPC], f32r, tag="fifths")
            nc.gpsimd.dma_start(fifths[:], fifths_d.ap().bitcast(f32r))
            bias_row_v = cpool.tile([1, FD], f32r, tag="bias_row_v")
            nc.gpsimd.dma_start(bias_row_v[:], bvis_d.ap().bitcast(f32r))
            bias_row_s = cpool.tile([1, SEM], f32r, tag="bias_row_s")
            nc.gpsimd.dma_start(bias_row_s[:], bsem_d.ap().bitcast(f32r))
            bm1T = cpool.tile([128, 3], f32, tag="bm1T")
            bm2T = cpool.tile([128, 3], f32, tag="bm2T")
            for c, (off, sz) in enumerate(SEMCH):
                nc.gpsimd.dma_start(bm1T[0:sz, c : c + 1], bm1_d.ap()[off : off + sz, :])
                nc.gpsimd.dma_start(bm2T[0:sz, c : c + 1], bm2_d.ap()[off : off + sz, :])
            wm1 = wres.tile([128, 3, SEM], f32, tag="wm1")
            wm2 = wres.tile([128, 3, SEM], f32, tag="wm2")
            for c, (off, sz) in enumerate(SEMCH):
                nc.gpsimd.dma_start(wm1[0:sz, c, :], wm1_d.ap()[off : off + sz, :])
                nc.gpsimd.dma_start(wm2[0:sz, c, :], wm2_d.ap()[off : off + sz, :])

            # helpers
            def ptranspose(in_ap, fast=False):
                p = in_ap.partition_size()
                f = in_ap.free_size()
                t = pt.tile([128, 512], f32, tag="tr")
                if fast:
                    nc.tensor.transpose(t[0:f, 0:p].bitcast(f32r), in_ap.bitcast(f32r),
                                        identr[0:p, 0:p])
                else:
                    nc.tensor.transpose(t[0:f, 0:p], in_ap.bitcast(f32), ident[0:p, 0:p])
                return t

            # grouped transposes: pack several [p,f]->[f,p] into ONE psum bank,
            # columns laid out back-to-back; returns (tile, col_offsets)
            def ptranspose_grp(in_aps, fast=False):
                t = pt.tile([128, 512], f32, tag="tr")
                offs = []
                col = 0
                for ia in in_aps:
                    p = ia.partition_size()
                    f = ia.free_size()
                    assert col + p <= 512
                    if fast:
                        nc.tensor.transpose(t[0:f, col : col + p].bitcast(f32r),
                                            ia.bitcast(f32r), identr[0:p, 0:p])
                    else:
                        nc.tensor.transpose(t[0:f, col : col + p], ia.bitcast(f32),
                                            ident[0:p, 0:p])
                    offs.append(col)
                    col += p
                return t, offs

            def copy_ps2(dst, src):
                fs = src.free_size()
                h = ((fs // 2) + 3) // 4 * 4
                nc.vector.tensor_copy(dst[:, 0:h], src[:, 0:h])
                nc.scalar.copy(dst[:, h:fs], src[:, h:fs])

            _ci = [0]
            def copy_ps(dst, src):
                _ci[0] += 1
                if _ci[0] % 2:
                    nc.vector.tensor_copy(dst, src)
                else:
                    nc.scalar.copy(dst, src)

            # accumulate a [M,1024] = sum_k lhsT_k.T @ rhs_k via two 1-bank halves.
            # chunks: list of (lhsT_ap, rhs_full_ap) with rhs [K,1024]
            def acc_1024(m, chunks, out_cb):
                ph0 = pacc.tile([B20, 512], f32, tag="pacc")
                ph1 = pacc.tile([B20, 512], f32, tag="pacc")
                ph = [ph0, ph1]
                n = len(chunks)
                for i, (l, r) in enumerate(chunks):
                    for h in range(2):
                        nc.tensor.matmul(ph[h][0:m, :], l, r[:, h * 512 : (h + 1) * 512],
                                         start=(i == 0), stop=(i == n - 1))
                for h in range(2):
                    out_cb(h, ph[h])

            # ================= sc/ss transposes + sMLP =================
            scT = apool.tile([128, FDC, B20], f32r, tag="scT")
            for g in range(2):
                t, _ = ptranspose_grp([sc_nat[:, (g * 4 + i) * 128 : (g * 4 + i + 1) * 128]
                                       for i in range(4)])
                copy_ps(scT[:, g * 4 : (g + 1) * 4, :], t[0:128, 0 : 4 * B20])
            ssT = apool.tile([128, 3, B20], f32, tag="ssT")
            t_ss, _ = ptranspose_grp([ss_nat[:, off : off + sz] for (off, sz) in SEMCH])
            copy_ps(ssT[:], t_ss[0:128, 0 : 3 * B20])

            h1T = apool.tile([128, 3, B20], f32, tag="h1T")
            for mc, (moff, msz) in enumerate(SEMCH):
                lk3 = npool_s.tile([128, B20], f32, tag="nat12")
                ph = psm.tile([128, B20], f32, tag="ps1")
                for kc, (koff, ksz) in enumerate(SEMCH):
                    nc.tensor.matmul(ph[0:msz, :], wm1[0:ksz, kc, moff : moff + msz],
                                     ssT[0:ksz, kc, :], start=(kc == 0), stop=(kc == 2))
                nc.vector.tensor_scalar(lk3[0:msz, :], ph[0:msz, :], bm1T[0:msz, mc : mc + 1],
                                        0.1, op0=ALU.add, op1=ALU.mult)
                nc.vector.tensor_scalar(h1T[0:msz, mc, :], ph[0:msz, :], bm1T[0:msz, mc : mc + 1],
                                        None, op0=ALU.add)
                nc.vector.tensor_tensor(h1T[0:msz, mc, :], h1T[0:msz, mc, :], lk3[0:msz, :],
                                        op=ALU.max)
            sT = apool.tile([128, 3, B20], f32r, tag="sT")
            for mc, (moff, msz) in enumerate(SEMCH):
                ph = psm.tile([128, B20], f32, tag="ps1")
                for kc, (koff, ksz) in enumerate(SEMCH):
                    nc.tensor.matmul(ph[0:msz, :], wm2[0:ksz, kc, moff : moff + msz],
                                     h1T[0:ksz, kc, :], start=(kc == 0), stop=(kc == 2))
                nc.vector.tensor_scalar(sT[0:msz, mc, :], ph[0:msz, :], bm2T[0:msz, mc : mc + 1],
                                        None, op0=ALU.add)

            qf_tiles = []
            for e in range(EPC):
                qt = qpool.tile([NQ, FD], f32, tag="qf_nat")
                nc.sync.dma_start(qt[:], qf_d.ap()[e])
                qf_tiles.append(qt)

            # ================= qf normalize + transpose (early) =================
            qnT_tiles = []
            for e in range(EPC):
                qt = qf_tiles[e]
                sq = sqpool.tile([NQ, FD], f32, tag="sq4k")
                ssq = smp.tile([NQ, 1], f32, tag="ssq_q")
                nc.scalar.activation(sq[:], qt[:], AF.Square, accum_out=ssq[:])
                rq = smp.tile([NQ, 1], f32, tag="rq_q")
                nc.vector.reciprocal(rq[:], ssq[:])
                s10 = smp.tile([NQ, 1], f32, tag="s10")
                nc.scalar.activation(s10[:], rq[:], AF.Sqrt, scale=float(temp) * float(temp))
                nc.vector.tensor_scalar(qt[:], qt[:], s10[:], None, op0=ALU.mult)
                qnT = qntp.tile([128, FDC, NQ], f32, tag="qnT")
                for g in range(2):
                    t, _ = ptranspose_grp([qt[:, (g * 4 + i) * 128 : (g * 4 + i + 1) * 128]
                                           for i in range(4)])
                    copy_ps(qnT[:, g * 4 : (g + 1) * 4, :], t[0:128, 0 : 4 * NQ])
                qnT_tiles.append(qnT)

            # ================= q = sc @ Wq + s @ Wqs =================
            q_chunks = []
            for dc in range(FDC):
                w = wbig.tile([128, FD], f32r, tag="wbig")
                nc.sync.dma_start(w[:], wq_d.ap()[dc * 128 : (dc + 1) * 128, :].bitcast(f32r))
                q_chunks.append((scT[:, dc, :], w[:, :]))
            for c, (off, sz) in enumerate(SEMCH):
                w = wbig.tile([128, FD], f32r, tag="wbig")
                nc.sync.dma_start(w[0:sz, :], wqs_d.ap()[off : off + sz, :].bitcast(f32r))
                q_chunks.append((sT[0:sz, c, :], w[0:sz, :]))
            q_nat = npool.tile([B20, FD], f32, tag="nat4k")
            acc_1024(B20, q_chunks,
                     lambda h, ph: nc.vector.tensor_copy(q_nat[:, h * 512 : (h + 1) * 512], ph[0:B20, :]))
            qT = apool.tile([128, FDC, B20], f32r, tag="qT")
            for g in range(2):
                t, _ = ptranspose_grp([q_nat[:, (g * 4 + i) * 128 : (g * 4 + i + 1) * 128]
                                       for i in range(4)])
                copy_ps(qT[:, g * 4 : (g + 1) * 4, :], t[0:128, 0 : 4 * B20])

            # ================= t1 = q @ Wk^T =================
            t1_chunks = []
            for kc in range(FDC):
                wt = wkc.tile([128, FDC, 128], f32, tag="wkcol")
                nc.sync.dma_start(wt[:], wk_d.ap()[:, kc * 128 : (kc + 1) * 128]
                                    .rearrange("(c p) n -> p c n", p=128))
                wkTc = wktp.tile([128, FD], f32r, tag="wkT")
                for g in range(2):
                    t, _ = ptranspose_grp([wt[:, g * 4 + i, :] for i in range(4)])
                    copy_ps(wkTc[:, g * 512 : (g + 1) * 512], t[0:128, :])
                t1_chunks.append((qT[:, kc, :], wkTc[:, :]))
            t1_nat = npool.tile([B20, FD], f32, tag="nat4k")
            acc_1024(B20, t1_chunks,
                     lambda h, ph: nc.vector.tensor_copy(t1_nat[:, h * 512 : (h + 1) * 512], ph[0:B20, :]))

            # ================= t2 = q @ Wks^T =================
            wks_nat = wres.tile([128, 3, FD], f32, tag="wks_nat")
            for c, (off, sz) in enumerate(SEMCH):
                nc.sync.dma_start(wks_nat[0:sz, c, :], wks_d.ap()[off : off + sz, :])
            pt2 = psm.tile([B20, SEM], f32, tag="ps1")
            for kc in range(FDC):
                wksTc = wsm.tile([128, SEM], f32r, tag="wksT")
                t, _ = ptranspose_grp([wks_nat[0:sz, c, kc * 128 : (kc + 1) * 128]
                                       for c, (off, sz) in enumerate(SEMCH)])
                copy_ps(wksTc[:], t[0:128, 0:SEM])
                nc.tensor.matmul(pt2[:], qT[:, kc, :], wksTc[:],
                                 start=(kc == 0), stop=(kc == FDC - 1))
            t2_nat = npool_s.tile([B20, SEM], f32, tag="nat12")
            nc.vector.tensor_copy(t2_nat[:], pt2[:])

            # ================= avg per episode =================
            avgvT = apool.tile([128, FDC, EPC], f32r, tag="avgvT")
            avgsT = apool.tile([128, 3, EPC], f32r, tag="avgsT")
            for e in range(EPC):
                avg_nat = npool.tile([1, FD], f32, tag="nat4k")
                acc_1024(1, [(inv512[:], bw_nat[e][:, c, :]) for c in range(NBC)],
                         lambda h, ph: nc.vector.tensor_copy(avg_nat[:, h * 512 : (h + 1) * 512], ph[0:1, :]))
                t, _ = ptranspose_grp([avg_nat[:, dc * 128 : (dc + 1) * 128] for dc in range(FDC)])
                nc.vector.tensor_copy(avgvT[:, :, e], t[0:128, 0:FDC])
                ps_ = psm.tile([1, SEM], f32, tag="ps1")
                for c in range(NBC):
                    nc.tensor.matmul(ps_[:], inv512[:], bsm_nat[e][:, c, :],
                                     start=(c == 0), stop=(c == NBC - 1))
                avgs_nat = npool_s.tile([1, SEM], f32, tag="nat12")
                nc.vector.tensor_copy(avgs_nat[:], ps_[:])
                t, _ = ptranspose_grp([avgs_nat[:, off : off + sz] for (off, sz) in SEMCH])
                nc.vector.tensor_copy(avgsT[:, :, e], t[0:128, 0:3])

            # ================= gates =================
            g_chunks = []
            for dc in range(FDC):
                w = wgate.tile([128, FD], f32r, tag="wgate")
                nc.sync.dma_start(w[:], wvis_d.ap()[dc * 128 : (dc + 1) * 128, :].bitcast(f32r))
                g_chunks.append((avgvT[:, dc, :], w[:, :]))
            for c, (off, sz) in enumerate(SEMCH):
                w = wgate.tile([128, FD], f32r, tag="wgate")
                nc.sync.dma_start(w[0:sz, :], wvis_d.ap()[FD + off : FD + off + sz, :].bitcast(f32r))
                g_chunks.append((avgsT[0:sz, c, :], w[0:sz, :]))
            g_chunks.append((one4[:], bias_row_v[:, :]))
            gpre_vis = npool.tile([EPC, FD], f32, tag="nat4k")
            acc_1024(EPC, g_chunks,
                     lambda h, ph: nc.vector.tensor_copy(gpre_vis[:, h * 512 : (h + 1) * 512], ph[0:EPC, :]))

            pgs = psm.tile([EPC, SEM], f32, tag="ps1")
            wsem_list = []
            for dc in range(FDC):
                w = wsm.tile([128, SEM], f32r, tag="wsem")
                nc.sync.dma_start(w[:], wsem_d.ap()[dc * 128 : (dc + 1) * 128, :].bitcast(f32r))
                wsem_list.append((avgvT[:, dc, :], w[0:128, :]))
            for c, (off, sz) in enumerate(SEMCH):
                w = wsm.tile([128, SEM], f32r, tag="wsem")
                nc.sync.dma_start(w[0:sz, :], wsem_d.ap()[FD + off : FD + off + sz, :].bitcast(f32r))
                wsem_list.append((avgsT[0:sz, c, :], w[0:sz, :]))
            wsem_list.append((one4[:], bias_row_s[:, :]))
            for i, (l, r) in enumerate(wsem_list):
                nc.tensor.matmul(pgs[:], l, r, start=(i == 0), stop=(i == len(wsem_list) - 1))
            gpre_sem = npool_s.tile([EPC, SEM], f32, tag="nat12")
            nc.vector.tensor_copy(gpre_sem[:], pgs[:])

            gvisT = apool.tile([128, FDC, EPC], f32, tag="gvisT")
            for g in range(2):
                t, _ = ptranspose_grp([gpre_vis[:, (g * 4 + i) * 128 : (g * 4 + i + 1) * 128]
                                       for i in range(4)])
                nc.scalar.activation(gvisT[:, g * 4 : (g + 1) * 4, :], t[0:128, 0 : 4 * EPC], AF.Sigmoid)
                nc.vector.tensor_scalar_add(gvisT[:, g * 4 : (g + 1) * 4, :],
                                            gvisT[:, g * 4 : (g + 1) * 4, :], 1.0)
            gsemT = apool.tile([128, 3, EPC], f32, tag="gsemT")
            t_gs, _ = ptranspose_grp([gpre_sem[:, off : off + sz] for (off, sz) in SEMCH])
            nc.scalar.activation(gsemT[:], t_gs[0:128, 0 : 3 * EPC], AF.Sigmoid)
            nc.vector.tensor_scalar_add(gsemT[:], gsemT[:], 1.0)

            # ================= gated projections t1g, t2g =================
            t1gT = apool.tile([128, FDC, B20], f32r, tag="t1gT")
            for g in range(2):
                t, offs = ptranspose_grp([t1_nat[:, (g * 4 + i) * 128 : (g * 4 + i + 1) * 128]
                                          for i in range(4)])
                for i in range(4):
                    dc = g * 4 + i
                    for e in range(EPC):
                        nc.vector.tensor_scalar(t1gT[:, dc, e * NW : (e + 1) * NW],
                                                t[0:128, offs[i] + e * NW : offs[i] + (e + 1) * NW],
                                                gvisT[:, dc, e : e + 1], None, op0=ALU.mult)
            t2gT = apool.tile([128, 3, B20], f32r, tag="t2gT")
            t2g_p, offs2 = ptranspose_grp([t2_nat[:, off : off + sz] for (off, sz) in SEMCH])
            for c, (off, sz) in enumerate(SEMCH):
                for e in range(EPC):
                    nc.vector.tensor_scalar(t2gT[0:sz, c, e * NW : (e + 1) * NW],
                                            t2g_p[0:sz, offs2[c] + e * NW : offs2[c] + (e + 1) * NW],
                                            gsemT[0:sz, c, e : e + 1], None, op0=ALU.mult)

            # ================= per-episode attention =================
            ugT = apool.tile([128, FDC, B20], f32r, tag="ugT")
            for e in range(EPC):
                bwt = bw_nat[e]
                bst = bsm_nat[e]
                psc = psm.tile([NW, NB], f32, tag="ps1")
                for dc in range(FDC):
                    stg = spool2.tile([128, NB], f32r, tag="bwT_st")
                    t, _ = ptranspose_grp([bwt[:, c4, dc * 128 : (dc + 1) * 128]
                                           for c4 in range(NBC)], fast=True)
                    copy_ps(stg[:], t[0:128, :])
                    nc.tensor.matmul(psc[:], t1gT[:, dc, e * NW : (e + 1) * NW], stg[:],
                                     start=(dc == 0), stop=False)
                for c, (off, sz) in enumerate(SEMCH):
                    stg = spool2.tile([128, NB], f32r, tag="bwT_st")
                    t, _ = ptranspose_grp([bst[:, c4, off : off + sz]
                                           for c4 in range(NBC)], fast=True)
                    copy_ps(stg[0:sz, :], t[0:sz, :])
                    nc.tensor.matmul(psc[:], t2gT[0:sz, c, e * NW : (e + 1) * NW], stg[0:sz, :],
                                     start=False, stop=(c == 2))

                mx = smp.tile([NW, 1], f32, tag="mx")
                nc.vector.reduce_max(mx[:], psc[:], axis=AX.X)
                mxn = smp.tile([NW, 1], f32, tag="mxn")
                nc.vector.tensor_scalar(mxn[:], mx[:], -1.0 / 32.0, None, op0=ALU.mult)
                attn = spool2.tile([NW, NB], f32, tag="attn")
                sm = smp.tile([NW, 1], f32, tag="sm")
                nc.scalar.activation(attn[:], psc[:], AF.Exp, bias=mxn[:], scale=1.0 / 32.0,
                                     accum_out=sm[:])
                rs = smp.tile([NW, 1], f32, tag="rs")
                nc.vector.reciprocal(rs[:], sm[:])
                nc.vector.tensor_scalar(attn[:], attn[:], rs[:], None, op0=ALU.mult)

                attnT = spool2.tile([128, NBC, NW], f32r, tag="attnT")
                t, _ = ptranspose_grp([attn[:, c4 * 128 : (c4 + 1) * 128] for c4 in range(NBC)])
                copy_ps(attnT[:], t[0:128, 0 : NBC * NW])

                u_nat = npool.tile([NW, FD], f32, tag="nat4k")
                acc_1024(NW, [(attnT[:, c4, :], bwt[:, c4, :]) for c4 in range(NBC)],
                         lambda h, ph: nc.vector.tensor_copy(u_nat[:, h * 512 : (h + 1) * 512], ph[0:NW, :]))
                for g in range(2):
                    t, offs = ptranspose_grp([u_nat[:, (g * 4 + i) * 128 : (g * 4 + i + 1) * 128]
                                              for i in range(4)])
                    for i in range(4):
                        dc = g * 4 + i
                        nc.vector.tensor_scalar(ugT[:, dc, e * NW : (e + 1) * NW],
                                                t[0:128, offs[i] : offs[i] + NW],
                                                gvisT[:, dc, e : e + 1], None, op0=ALU.mult)

            # ================= out = ug @ Wv ; out2 = out @ Wfc + sc =================
            _park = [(wgate, "wgate"), (wbig, "wbig"), (wktp, "wkT"), (wkc, "wkcol"),
                     (wlt, "wlate")]
            o_chunks = []
            for dc in range(FDC):
                pool_, tag_ = _park[dc % 4]
                w = pool_.tile([128, FD], f32r, tag=tag_)
                nc.sync.dma_start(w[:], wv_d.ap()[dc * 128 : (dc + 1) * 128, :].bitcast(f32r))
                o_chunks.append((ugT[:, dc, :], w[:, :]))
            out_nat = npool.tile([B20, FD], f32, tag="nat4k")
            acc_1024(B20, o_chunks,
                     lambda h, ph: nc.vector.tensor_copy(out_nat[:, h * 512 : (h + 1) * 512], ph[0:B20, :]))
            outT = apool.tile([128, FDC, B20], f32r, tag="outT")
            for g in range(2):
                t, _ = ptranspose_grp([out_nat[:, (g * 4 + i) * 128 : (g * 4 + i + 1) * 128]
                                       for i in range(4)])
                copy_ps(outT[:, g * 4 : (g + 1) * 4, :], t[0:128, 0 : 4 * B20])

            o2_chunks = []
            for dc in range(FDC):
                pool_, tag_ = _park[(dc + 2) % 5]
                w = pool_.tile([128, FD], f32r, tag=tag_)
                nc.sync.dma_start(w[:], wfc_d.ap()[dc * 128 : (dc + 1) * 128, :].bitcast(f32r))
                o2_chunks.append((outT[:, dc, :], w[:, :]))
            out2 = apool.tile([B20, FD], f32r, tag="out2")
            acc_1024(B20, o2_chunks,
                     lambda h, ph: nc.vector.tensor_tensor(out2[:, h * 512 : (h + 1) * 512], ph[0:B20, :],
                                                           sc_nat[:, h * 512 : (h + 1) * 512], op=ALU.add))

            # ================= fake + normalize + pnT =================
            pn_fk = npool.tile([EPC, FD], f32, tag="nat4k")
            ssf = smp.tile([EPC, 1], f32, tag="ssf")
            sqp = sqpool.tile([NQ, FD], f32, tag="sq4k")

            def fake_half(h, ph):
                nc.vector.tensor_copy(pn_fk[:, h * 512 : (h + 1) * 512], ph[0:EPC, :])
            acc_1024(EPC, [(fifths[:], out2[:, :])], fake_half)

            ssq = smp.tile([B20, 1], f32, tag="ssq")
            nc.scalar.activation(sqp[0:B20, :], sc_nat[:], AF.Square, accum_out=ssq[:])
            rqv = smp.tile([B20, 1], f32, tag="rq")
            nc.vector.reciprocal(rqv[:], ssq[:])
            inv_sc = smp.tile([B20, 1], f32, tag="inv_sc")
            nc.scalar.activation(inv_sc[:], rqv[:], AF.Sqrt)
            pn_sc = sqpool.tile([B20, FD], f32, tag="sq4k")
            nc.vector.tensor_scalar(pn_sc[:], sc_nat[:], inv_sc[:], None, op0=ALU.mult)

            nc.scalar.activation(sqp[0:EPC, :], pn_fk[:], AF.Square, accum_out=ssf[:])
            rf = smp.tile([EPC, 1], f32, tag="rf")
            nc.vector.reciprocal(rf[:], ssf[:])
            inv_f = smp.tile([EPC, 1], f32, tag="inv_f")
            nc.scalar.activation(inv_f[:], rf[:], AF.Sqrt)
            nc.vector.tensor_scalar(pn_fk[:], pn_fk[:], inv_f[:], None, op0=ALU.mult)

            pnT = apool.tile([128, FDC, EPC * NPROTO], f32, tag="pnT")
            for dc in range(FDC):
                t, offs = ptranspose_grp([pn_sc[:, dc * 128 : (dc + 1) * 128],
                                          pn_fk[:, dc * 128 : (dc + 1) * 128]])
                dst = pnT[:, dc, :].rearrange("p (e s) -> p e s", s=NPROTO)[:, :, 0:NW]
                srcp = t[0:128, 0:B20].rearrange("p (e w) -> p e w", w=NW)
                nc.vector.tensor_copy(dst, srcp)
                dst2 = pnT[:, dc, :].rearrange("p (e s) -> p e s", s=NPROTO)[:, :, NW]
                nc.vector.tensor_copy(dst2, t[0:128, offs[1] : offs[1] + EPC])

            # ================= logits =================
            for e in range(EPC):
                pl = psm.tile([NQ, NPROTO], f32, tag="ps1")
                for dc in range(FDC):
                    nc.tensor.matmul(pl[:], qnT_tiles[e][:, dc, :],
                                     pnT[:, dc, e * NPROTO : (e + 1) * NPROTO],
                                     start=(dc == 0), stop=(dc == FDC - 1))
                lg = smp.tile([NQ, NPROTO], f32, tag="lg")
                nc.vector.tensor_copy(lg[:], pl[:])
                nc.gpsimd.dma_start(out_d.ap()[e], lg[:])

    nc.finalize()
    return nc


def _aux_inputs():
    ident = np.eye(128, dtype=np.float32)
    inv512 = np.full((128, 1), 1.0 / 512.0, dtype=np.float32)
    one4 = np.ones((1, EPC), dtype=np.float32)
    fifths = np.zeros((B20, EPC), dtype=np.float32)
    for e in range(EPC):
        fifths[e * NW : (e + 1) * NW, e] = 1.0 / NW
    return {
        "aux_ident": ident,
        "aux_inv512": inv512,
        "aux_one4": one4,
        "aux_fifths": fifths,
    }


def kernel(**inputs):
    from concourse.bass_utils import run_bass_kernel_spmd

    temp = float(np.asarray(inputs["temp"]))
    key = ("v3", temp)
    if key not in _MODULE_CACHE:
        _MODULE_CACHE[key] = _build_module(temp)
    nc = _MODULE_CACHE[key]

    aux = _aux_inputs()
    per_ep = ["support_center", "base_weights", "support_seman", "base_seman", "query_feature"]
    weights = ["Wm1", "bm1", "Wm2", "bm2", "Wvis", "bvis", "Wsem", "bsem",
               "Wq", "Wk", "Wv", "Wqs", "Wks", "Wfc"]
    in_maps = []
    for c in range(NCORES):
        m = {}
        for k in per_ep:
            m[k] = np.ascontiguousarray(np.asarray(inputs[k])[c * EPC : (c + 1) * EPC])
        for k in weights:
            a = np.ascontiguousarray(np.asarray(inputs[k], dtype=np.float32))
            if k in ("bm1", "bm2"):
                a = a.reshape(SEM, 1)
            elif k == "bvis":
                a = a.reshape(1, FD)
            elif k == "bsem":
                a = a.reshape(1, SEM)
            m[k] = a
        m.update(aux)
        in_maps.append(m)

    res = run_bass_kernel_spmd(nc, in_maps, core_ids=list(range(NCORES)))
    out = np.concatenate([res.results[c]["out"] for c in range(NCORES)], axis=0)
    return out.astype(np.float32)



# revision 7
# speedup vs baseline: 1.8546x; 1.8546x over previous
"""Trainium2 Bass kernel for nn_Classifier_22625887715977 (sparse_attention).

kernel(**inputs) takes FULL unsharded inputs (bs=32) and returns the full
[32, 75, 6] logits. Shards the batch over 8 NeuronCores (4 episodes per
core); weights replicated, converted to bf16 on host, streamed.

Math (per episode, exact reassociation of the reference — never materializes
the expanded per-(episode,way) base bank):
  s      = leaky(ss @ Wm1 + bm1) @ Wm2 + bm2
  avg    = mean_n [bw | bsm]
  gvis   = sigmoid(avg @ Wvis + bvis) + 1 ; gsem = sigmoid(avg @ Wsem + bsem) + 1
  q      = sc @ Wq + s @ Wqs
  scores = ((q @ Wk^T) * gvis) @ bw^T + ((q @ Wks^T) * gsem) @ bsm^T ; attn = softmax(scores/32)
  out    = ((attn @ bw) * gvis) @ Wv ; out2 = out @ Wfc + sc
  fake   = mean_w out2 ; protos = [sc; fake] ; logits = temp * cos(qf, protos)

v4 implementation notes:
 - All large streams (weights, banks, qf) are converted to bf16 on the host;
   matmuls run bf16 x bf16 -> f32 PSUM. The residual / cosine path stays f32.
 - Wk, Wks, sc, ss are host-TRANSPOSED so every matmul has its contraction
   dim on partitions naturally; projection outputs are computed directly in
   transposed form (qT, t1T, t2T, uT, outT) with tiny free sizes.
 - Gates are computed way-replicated ([B20, .] via an episode-selector lhsT)
   so gate application is a single elementwise multiply in transposed layout.
 - Three DMA queues: weights on sync (HWDGE/SP), banks on gpsimd (SWDGE),
   smalls + qf + output on scalar (HWDGE/ACT). Wv/Wfc stream last.
"""

import numpy as np
import ml_dtypes

BF16 = ml_dtypes.bfloat16

BS = 32
NCORES = 8
EPC = BS // NCORES          # 4 episodes per core
NW = 5
B20 = EPC * NW              # 20
FD = 1024
FDC = FD // 128             # 8
SEM = 300
SEMCH = [(0, 128), (128, 128), (256, 44)]
SEMP = 384                  # SEM padded to 3*128
NB = 512
NBC = NB // 128             # 4
NQ = 75
NPROTO = NW + 1             # 6
VIN = FD + SEM              # 1324
VINC = 11                   # ceil(1324/128)
VINP = VINC * 128           # 1408

_MODULE_CACHE = {}


def _build_module(temp: float):
    import concourse.bass as bass
    import concourse.mybir as mybir
    import concourse.tile as tile
    from concourse import bacc

    f32 = mybir.dt.float32
    f32r = mybir.dt.float32r
    bf = mybir.dt.bfloat16
    AF = mybir.ActivationFunctionType
    ALU = mybir.AluOpType
    AX = mybir.AxisListType

    nc = bacc.Bacc("TRN2", target_bir_lowering=False, debug=False)

    def di(name, shape, dt=f32):
        return nc.dram_tensor(name, shape, dt, kind="ExternalInput")

    # --- small / f32 inputs ---
    sc_d = di("sc_nat", [B20, FD])
    fifths_d = di("fifths", [B20, EPC])
    bm1_d = di("bm1c", [128, 3])
    bm2_d = di("bm2c", [128, 3])
    # --- bf16 smalls ---
    scT_d = di("scT", [FD, B20], bf)
    ssT_d = di("ssT", [3, 128, B20], bf)
    esel_d = di("esel", [EPC, 128, B20], bf)
    ones_d = di("ones20", [1, B20], bf)
    bvis_d = di("bvis_row", [1, FD], bf)
    bsem_d = di("bsem_row", [1, SEM], bf)
    ident_d = di("ident_bf", [128, 128], bf)
    # --- bf16 weights ---
    wm1_d = di("Wm1_pad", [SEMP, SEM], bf)
    wm2_d = di("Wm2_pad", [SEMP, SEM], bf)
    wq_d = di("Wq", [FD, FD], bf)
    wqs_d = di("Wqs_pad", [SEMP, FD], bf)
    wkT_d = di("WkT", [FD, FD], bf)
    wksT_d = di("WksT", [FD, SEM], bf)
    wv_d = di("Wv", [FD, FD], bf)
    wfc_d = di("Wfc", [FD, FD], bf)
    wvis_d = di("Wvis_pad", [VINP, FD], bf)
    wsem_d = di("Wsem_pad", [VINP, SEM], bf)
    # --- bf16 big activations ---
    bw_d = di("bw", [EPC, NB, FD], bf)
    bsm_d = di("bsm", [EPC, NB, SEM], bf)
    qf_d = di("qf", [EPC, NQ, FD], bf)
    out_d = nc.dram_tensor("out", [EPC, NQ, NPROTO], f32, kind="ExternalOutput")

    from contextlib import ExitStack
    with tile.TileContext(nc) as tc, ExitStack() as _ctx:
        def _pool(**kw):
            return _ctx.enter_context(tc.tile_pool(**kw))

        cpool = _pool(name="const", bufs=1)
        wres = _pool(name="wres", bufs=1)       # resident small weights
        wbig = _pool(name="wbig", bufs=2)       # Wq,WkT then Wv,Wfc reuse
        wvp = _pool(name="wvis", bufs=1)
        bpool = _pool(name="banks", bufs=1)     # bw / bsm natural (per-e tags)
        tpool = _pool(name="bankT", bufs=1)     # bwT / bsmT (per-e tags)
        apool = _pool(name="acts", bufs=1)      # long-lived activations
        spool = _pool(name="small", bufs=1)     # single-use scratch
        epool = _pool(name="ep", bufs=2)        # per-episode pipelined smalls
        qpool = _pool(name="qn", bufs=1)
        pt = _pool(name="pt", bufs=2, space="PSUM")
        pbig = _pool(name="pbig", bufs=2, space="PSUM")
        psm = _pool(name="psm", bufs=3, space="PSUM")

        # ---------------- DMA issue ----------------
        # scalar queue: smalls then qf
        ident = cpool.tile([128, 128], bf, tag="ident")
        nc.scalar.dma_start(ident[:], ident_d.ap())
        scT = cpool.tile([128, FDC, B20], bf, tag="scT")
        nc.scalar.dma_start(scT[:], scT_d.ap().rearrange("(c p) b -> p c b", p=128))
        ssT = cpool.tile([128, 3, B20], bf, tag="ssT")
        nc.scalar.dma_start(ssT[:], ssT_d.ap().rearrange("c p b -> p c b"))
        sc_nat = cpool.tile([B20, FD], f32, tag="sc_nat")
        nc.scalar.dma_start(sc_nat[:], sc_d.ap())
        esel = cpool.tile([128, EPC, B20], bf, tag="esel")
        nc.scalar.dma_start(esel[:], esel_d.ap().rearrange("e p b -> p e b"))
        fifths = cpool.tile([B20, EPC], f32, tag="fifths")
        nc.scalar.dma_start(fifths[:], fifths_d.ap())
        ones20 = cpool.tile([1, B20], bf, tag="ones20")
        nc.scalar.dma_start(ones20[:], ones_d.ap())
        bm1c = cpool.tile([128, 3], f32, tag="bm1c")
        nc.scalar.dma_start(bm1c[:], bm1_d.ap())
        bm2c = cpool.tile([128, 3], f32, tag="bm2c")
        nc.scalar.dma_start(bm2c[:], bm2_d.ap())
        bvis_row = cpool.tile([1, FD], bf, tag="bvis")
        nc.scalar.dma_start(bvis_row[:], bvis_d.ap())
        bsem_row = cpool.tile([1, SEM], bf, tag="bsem")
        nc.scalar.dma_start(bsem_row[:], bsem_d.ap())
        wm1 = wres.tile([128, 3, SEM], bf, tag="wm1")
        nc.scalar.dma_start(wm1[:], wm1_d.ap().rearrange("(c p) d -> p c d", p=128))
        wm2 = wres.tile([128, 3, SEM], bf, tag="wm2")
        nc.scalar.dma_start(wm2[:], wm2_d.ap().rearrange("(c p) d -> p c d", p=128))
        # qf loaded per-episode (cycling buffer) in the qnT loop below

        # sync queue: big weights in need-order
        wq = wbig.tile([128, FDC, FD], bf, tag="wbig")
        nc.sync.dma_start(wq[:], wq_d.ap().rearrange("(c p) d -> p c d", p=128))
        wqs = wres.tile([128, 3, FD], bf, tag="wqs")
        nc.sync.dma_start(wqs[:], wqs_d.ap().rearrange("(c p) d -> p c d", p=128))
        wkT = wbig.tile([128, FDC, FD], bf, tag="wbig")
        nc.sync.dma_start(wkT[:], wkT_d.ap().rearrange("(c p) d -> p c d", p=128))
        wksT = wres.tile([128, FDC, SEM], bf, tag="wksT")
        nc.sync.dma_start(wksT[:], wksT_d.ap().rearrange("(c p) d -> p c d", p=128))
        wvis = wvp.tile([128, VINC, FD], bf, tag="wvis")
        nc.sync.dma_start(wvis[:], wvis_d.ap().rearrange("(c p) d -> p c d", p=128))
        wsem = wvp.tile([128, VINC, SEM], bf, tag="wsem")
        nc.sync.dma_start(wsem[:], wsem_d.ap().rearrange("(c p) d -> p c d", p=128))
        wv = wbig.tile([128, FDC, FD], bf, tag="wbig")
        nc.sync.dma_start(wv[:], wv_d.ap().rearrange("(c p) d -> p c d", p=128))
        wfc = wbig.tile([128, FDC, FD], bf, tag="wbig")
        for h in range(2):
            nc.sync.dma_start(
                wfc[:, :, h * 512 : (h + 1) * 512],
                wfc_d.ap()[:, h * 512 : (h + 1) * 512]
                .rearrange("(c p) d -> p c d", p=128))

        # gpsimd queue: banks, episode-interleaved
        bw_nat, bsm_nat = [], []
        for e in range(EPC):
            bwt = bpool.tile([128, NBC, FD], bf, tag=f"bw{e}")
            nc.gpsimd.dma_start(bwt[:], bw_d.ap()[e].rearrange("(c p) d -> p c d", p=128))
            bw_nat.append(bwt)
            bst = bpool.tile([128, NBC, SEM], bf, tag=f"bsm{e}")
            nc.gpsimd.dma_start(bst[:], bsm_d.ap()[e].rearrange("(c p) d -> p c d", p=128))
            bsm_nat.append(bst)

        # memset partial-chunk tiles that are read at full 128 partitions
        h1T = apool.tile([128, 3, B20], bf, tag="h1T")
        sT = apool.tile([128, 3, B20], bf, tag="sT")
        nc.vector.memset(h1T[:], 0.0)
        nc.vector.memset(sT[:], 0.0)
        avgsT = apool.tile([128, 3, B20], bf, tag="avgsT")
        nc.vector.memset(avgsT[:], 0.0)

        # round-robin copy engines for PSUM->SBUF traffic
        _eng = [nc.vector, nc.scalar, nc.gpsimd]
        _ei = [0]

        def copy_rr(dst, src):
            e = _eng[_ei[0] % 3]
            _ei[0] += 1
            if e is nc.scalar:
                e.copy(dst, src)
            else:
                e.tensor_copy(dst, src)

        # ---------------- sMLP: s = leaky(ss@Wm1+bm1)@Wm2 + bm2 ----------------
        for mc, (moff, msz) in enumerate(SEMCH):
            ph = psm.tile([128, B20], f32, tag="ps_sm")
            for kc, (koff, ksz) in enumerate(SEMCH):
                nc.tensor.matmul(ph[0:msz, :], wm1[0:ksz, kc, moff : moff + msz],
                                 ssT[0:ksz, kc, :], start=(kc == 0), stop=(kc == 2))
            lk = spool.tile([128, B20], f32, tag="mlp_lk")
            nc.vector.tensor_scalar(lk[0:msz, :], ph[0:msz, :], bm1c[0:msz, mc : mc + 1],
                                    0.1, op0=ALU.add, op1=ALU.mult)
            nc.vector.tensor_scalar(h1T[0:msz, mc, :], ph[0:msz, :],
                                    bm1c[0:msz, mc : mc + 1], None, op0=ALU.add)
            nc.vector.tensor_tensor(h1T[0:msz, mc, :], h1T[0:msz, mc, :], lk[0:msz, :],
                                    op=ALU.max)
        for mc, (moff, msz) in enumerate(SEMCH):
            ph = psm.tile([128, B20], f32, tag="ps_sm")
            for kc, (koff, ksz) in enumerate(SEMCH):
                nc.tensor.matmul(ph[0:msz, :], wm2[0:ksz, kc, moff : moff + msz],
                                 h1T[0:ksz, kc, :], start=(kc == 0), stop=(kc == 2))
            nc.vector.tensor_scalar(sT[0:msz, mc, :], ph[0:msz, :],
                                    bm2c[0:msz, mc : mc + 1], None, op0=ALU.add)

        # ---------------- qT = (sc@Wq + s@Wqs)^T directly ----------------
        qT_ps = psm.tile([128, FDC, B20], f32, tag="ps_sm")
        for m in range(FDC):
            for kc in range(FDC):
                nc.tensor.matmul(qT_ps[:, m, :], wq[:, kc, m * 128 : (m + 1) * 128],
                                 scT[:, kc, :], start=(kc == 0), stop=False)
            for c in range(3):
                nc.tensor.matmul(qT_ps[:, m, :], wqs[:, c, m * 128 : (m + 1) * 128],
                                 sT[:, c, :], start=False, stop=(c == 2))
        qT = apool.tile([128, FDC, B20], bf, tag="qT")
        nc.vector.tensor_copy(qT[:], qT_ps[:])

        # ---------------- bank transposes (as banks land) + avg ----------------
        bwT_l, bsmT_l = [], []
        avgv_ps0 = pbig.tile([B20, 512], f32, tag="ps_big")
        avgv_ps1 = pbig.tile([B20, 512], f32, tag="ps_big")
        avgv_ps = [avgv_ps0, avgv_ps1]
        avgs_ps = psm.tile([B20, SEM], f32, tag="ps_sm")
        for e in range(EPC):
            # avg-vis accumulation chunks for this episode
            for c in range(NBC):
                for h in range(2):
                    nc.tensor.matmul(avgv_ps[h][:, :], esel[:, e, :],
                                     bw_nat[e][:, c, h * 512 : (h + 1) * 512],
                                     start=(e == 0 and c == 0),
                                     stop=(e == EPC - 1 and c == NBC - 1))
            # bwT: 8 dchunks x 4 nchunks, packed 2 dchunks per psum bank
            bwT = tpool.tile([128, FDC, NB], bf, tag=f"bwT{e}")
            for g in range(4):
                t = pt.tile([128, 1024], bf, tag="tr")
                for i in range(2):
                    dc = g * 2 + i
                    for c in range(NBC):
                        nc.tensor.transpose(
                            t[:, i * 512 + c * 128 : i * 512 + (c + 1) * 128],
                            bw_nat[e][:, c, dc * 128 : (dc + 1) * 128],
                            ident[:])
                copy_rr(bwT[:, g * 2 : g * 2 + 2, :], t[:])
            bwT_l.append(bwT)
            # avg-sem accumulation
            for c in range(NBC):
                nc.tensor.matmul(avgs_ps[:, :], esel[:, e, :], bsm_nat[e][:, c, :],
                                 start=(e == 0 and c == 0),
                                 stop=(e == EPC - 1 and c == NBC - 1))
            # bsmT: 3 semchunks x 4 nchunks -> [128, 3, 512]
            bsmT = tpool.tile([128, 3, NB], bf, tag=f"bsmT{e}")
            t2p = pt.tile([128, 1024], bf, tag="tr")
            for sci, (soff, ssz) in enumerate(SEMCH[:2]):
                for c in range(NBC):
                    nc.tensor.transpose(
                        t2p[:, sci * 512 + c * 128 : sci * 512 + (c + 1) * 128],
                        bsm_nat[e][:, c, soff : soff + ssz], ident[:])
            copy_rr(bsmT[:, 0:2, :], t2p[:])
            t3p = pt.tile([128, 1024], bf, tag="tr")
            soff, ssz = SEMCH[2]
            for c in range(NBC):
                nc.tensor.transpose(t3p[0:ssz, c * 128 : (c + 1) * 128],
                                    bsm_nat[e][:, c, soff : soff + ssz],
                                    ident[:])
            copy_rr(bsmT[0:ssz, 2, :], t3p[0:ssz, 0:512])
            bsmT_l.append(bsmT)

        # ---------------- qf normalize + qnT (early, under DMA) ----------------
        qnT_l = []
        for e in range(EPC):
            qf_e = epool.tile([NQ, FD], bf, tag="qf")
            nc.scalar.dma_start(qf_e[:], qf_d.ap()[e])
            ssq = epool.tile([NQ, 1], f32, tag="q_ssq")
            sq = spool.tile([NQ, FD], bf, tag="scratch4k")
            nc.scalar.activation(sq[:], qf_e[:], AF.Square, accum_out=ssq[:])
            rq = epool.tile([NQ, 1], f32, tag="q_rq")
            nc.vector.reciprocal(rq[:], ssq[:])
            s10 = epool.tile([NQ, 1], f32, tag="q_s10")
            nc.scalar.activation(s10[:], rq[:], AF.Sqrt, scale=float(temp) * float(temp))
            qn = epool.tile([NQ, FD], bf, tag="q_qn")
            nc.vector.tensor_scalar(qn[:], qf_e[:], s10[:], None, op0=ALU.mult)
            qnT = qpool.tile([128, FDC, NQ], bf, tag=f"qnT{e}")
            for g in range(2):
                tfull = pt.tile([128, 1024], bf, tag="tr")
                t = tfull[:, 0 : 4 * NQ]
                for i in range(4):
                    dc = g * 4 + i
                    nc.tensor.transpose(t[:, i * NQ : (i + 1) * NQ],
                                        qn[:, dc * 128 : (dc + 1) * 128], ident[0:NQ, 0:NQ])
                copy_rr(qnT[:, g * 4 : (g + 1) * 4, :], t[:])
            qnT_l.append(qnT)

        # ---------------- pn_sc = sc/||sc|| (early) + pnT fill ----------------
        ssq_sc = spool.tile([B20, 1], f32, tag="sc_ssq")
        sq_sc = spool.tile([NQ, FD], bf, tag="scratch4k")
        nc.scalar.activation(sq_sc[0:B20, :], sc_nat[:], AF.Square, accum_out=ssq_sc[:])
        r_sc = spool.tile([B20, 1], f32, tag="sc_r")
        nc.vector.reciprocal(r_sc[:], ssq_sc[:])
        inv_sc = spool.tile([B20, 1], f32, tag="sc_inv")
        nc.scalar.activation(inv_sc[:], r_sc[:], AF.Sqrt)
        pn_sc = spool.tile([B20, FD], bf, tag="pn_sc")
        nc.vector.tensor_scalar(pn_sc[:], sc_nat[:], inv_sc[:], None, op0=ALU.mult)
        pnT = apool.tile([128, FDC, EPC * NPROTO], bf, tag="pnT")
        for g in range(2):
            tfull = pt.tile([128, 1024], bf, tag="tr")
            t = tfull[:, 0 : 4 * B20]
            for i in range(4):
                dc = g * 4 + i
                nc.tensor.transpose(t[:, i * B20 : (i + 1) * B20],
                                    pn_sc[:, dc * 128 : (dc + 1) * 128], ident[0:B20, 0:B20])
            for i in range(4):
                dc = g * 4 + i
                dst = pnT[:, dc, :].rearrange("p (e s) -> p e s", s=NPROTO)[:, :, 0:NW]
                src = t[:, i * B20 : (i + 1) * B20].rearrange("p (e w) -> p e w", w=NW)
                nc.vector.tensor_copy(dst, src)
        # ---------------- t1T/t2T (needs WkT/WksT) ----------------
        t1_ps = psm.tile([128, FDC, B20], f32, tag="ps_sm")
        for m in range(FDC):
            for kc in range(FDC):
                nc.tensor.matmul(t1_ps[:, m, :], wkT[:, kc, m * 128 : (m + 1) * 128],
                                 qT[:, kc, :], start=(kc == 0), stop=(kc == FDC - 1))
        t2_ps = psm.tile([128, 3, B20], f32, tag="ps_sm")
        for mc, (moff, msz) in enumerate(SEMCH):
            for kc in range(FDC):
                nc.tensor.matmul(t2_ps[0:msz, mc, :], wksT[:, kc, moff : moff + msz],
                                 qT[:, kc, :], start=(kc == 0), stop=(kc == FDC - 1))

        # ---------------- avg transposes -> avgT (way-replicated) ----------------
        avgv_nat = spool.tile([B20, FD], bf, tag="avgv_nat")
        for h in range(2):
            nc.vector.tensor_copy(avgv_nat[:, h * 512 : (h + 1) * 512], avgv_ps[h][:, :])
        avgs_nat = spool.tile([B20, SEM], bf, tag="avgs_nat")
        nc.vector.tensor_copy(avgs_nat[:], avgs_ps[:])
        avgvT = apool.tile([128, FDC, B20], bf, tag="avgvT")
        for g in range(2):
            tfull = pt.tile([128, 1024], bf, tag="tr")
            t = tfull[:, 0 : 4 * B20]
            for i in range(4):
                dc = g * 4 + i
                nc.tensor.transpose(t[:, i * B20 : (i + 1) * B20],
                                    avgv_nat[:, dc * 128 : (dc + 1) * 128],
                                    ident[0:B20, 0:B20])
            copy_rr(avgvT[:, g * 4 : (g + 1) * 4, :], t[:])
        tsp_f = pt.tile([128, 1024], bf, tag="tr")
        tsp = tsp_f[:, 0 : 3 * B20]
        for sci, (soff, ssz) in enumerate(SEMCH):
            nc.tensor.transpose(tsp[0:ssz, sci * B20 : (sci + 1) * B20],
                                avgs_nat[:, soff : soff + ssz], ident[0:B20, 0:B20])
        nc.vector.tensor_copy(avgsT[0:128, 0, :], tsp[0:128, 0:B20])
        nc.vector.tensor_copy(avgsT[0:128, 1, :], tsp[0:128, B20 : 2 * B20])
        soff, ssz = SEMCH[2]
        nc.vector.tensor_copy(avgsT[0:ssz, 2, :], tsp[0:ssz, 2 * B20 : 3 * B20])

        # ---------------- gates ----------------
        gpv_ps0 = pbig.tile([B20, 512], f32, tag="ps_big")
        gpv_ps1 = pbig.tile([B20, 512], f32, tag="ps_big")
        gpv_ps = [gpv_ps0, gpv_ps1]
        nchunks = VINC + 1
        for h in range(2):
            for kc in range(FDC):
                nc.tensor.matmul(gpv_ps[h][:, :], avgvT[:, kc, :],
                                 wvis[:, kc, h * 512 : (h + 1) * 512],
                                 start=(kc == 0), stop=False)
            for c in range(3):
                nc.tensor.matmul(gpv_ps[h][:, :], avgsT[:, c, :],
                                 wvis[:, FDC + c, h * 512 : (h + 1) * 512],
                                 start=False, stop=False)
            nc.tensor.matmul(gpv_ps[h][:, :], ones20[:], bvis_row[:, h * 512 : (h + 1) * 512],
                             start=False, stop=True)
        gps_ps = psm.tile([B20, SEM], f32, tag="ps_sm")
        for kc in range(FDC):
            nc.tensor.matmul(gps_ps[:, :], avgvT[:, kc, :], wsem[:, kc, :],
                             start=(kc == 0), stop=False)
        for c in range(3):
            nc.tensor.matmul(gps_ps[:, :], avgsT[:, c, :], wsem[:, FDC + c, :],
                             start=False, stop=False)
        nc.tensor.matmul(gps_ps[:, :], ones20[:], bsem_row[:], start=False, stop=True)

        gv_nat = spool.tile([B20, FD], bf, tag="avgv_nat")
        for h in range(2):
            nc.scalar.activation(gv_nat[:, h * 512 : (h + 1) * 512], gpv_ps[h][:, :],
                                 AF.Sigmoid)
        gs_nat = spool.tile([B20, SEM], bf, tag="avgs_nat")
        nc.scalar.activation(gs_nat[:], gps_ps[:], AF.Sigmoid)

        gvis5T = apool.tile([128, FDC, B20], bf, tag="gvis5T")
        for g in range(2):
            tfull = pt.tile([128, 1024], bf, tag="tr")
            t = tfull[:, 0 : 4 * B20]
            for i in range(4):
                dc = g * 4 + i
                nc.tensor.transpose(t[:, i * B20 : (i + 1) * B20],
                                    gv_nat[:, dc * 128 : (dc + 1) * 128],
                                    ident[0:B20, 0:B20])
            nc.vector.tensor_scalar_add(gvis5T[:, g * 4 : (g + 1) * 4, :], t[:], 1.0)
        gsem5T = apool.tile([128, 3, B20], bf, tag="gsem5T")
        tg_f = pt.tile([128, 1024], bf, tag="tr")
        tg = tg_f[:, 0 : 3 * B20]
        for sci, (soff, ssz) in enumerate(SEMCH):
            nc.tensor.transpose(tg[0:ssz, sci * B20 : (sci + 1) * B20],
                                gs_nat[:, soff : soff + ssz], ident[0:B20, 0:B20])
        nc.vector.tensor_scalar_add(gsem5T[0:128, 0:2, :],
                                    tg[0:128, 0 : 2 * B20].rearrange("p (c b) -> p c b", c=2),
                                    1.0)
        soff, ssz = SEMCH[2]
        nc.vector.tensor_scalar_add(gsem5T[0:ssz, 2, :], tg[0:ssz, 2 * B20 : 3 * B20], 1.0)

        # gated projections
        t1gT = apool.tile([128, FDC, B20], bf, tag="t1gT")
        t1c = spool.tile([128, FDC, B20], bf, tag="t1c")
        nc.vector.tensor_copy(t1c[:], t1_ps[:])
        nc.vector.tensor_tensor(t1gT[:], t1c[:], gvis5T[:], op=ALU.mult)
        t2gT = apool.tile([128, 3, B20], bf, tag="t2gT")
        t2c = spool.tile([128, 3, B20], bf, tag="t2c")
        nc.vector.tensor_copy(t2c[:], t2_ps[:])
        nc.vector.tensor_tensor(t2gT[:], t2c[:], gsem5T[:], op=ALU.mult)

        # ---------------- per-episode attention ----------------
        ugT = apool.tile([128, FDC, B20], bf, tag="ugT")
        for e in range(EPC):
            sc_ps = psm.tile([NW, NB], f32, tag="ps_sm")
            for dc in range(FDC):
                nc.tensor.matmul(sc_ps[:], t1gT[:, dc, e * NW : (e + 1) * NW],
                                 bwT_l[e][:, dc, :], start=(dc == 0), stop=False)
            for sci, (soff, ssz) in enumerate(SEMCH):
                nc.tensor.matmul(sc_ps[:], t2gT[0:ssz, sci, e * NW : (e + 1) * NW],
                                 bsmT_l[e][0:ssz, sci, :], start=False, stop=(sci == 2))
            mx = epool.tile([NW, 1], f32, tag="mx")
            nc.vector.reduce_max(mx[:], sc_ps[:], axis=AX.X)
            mxn = epool.tile([NW, 1], f32, tag="mxn")
            nc.vector.tensor_scalar(mxn[:], mx[:], -1.0 / 32.0, None, op0=ALU.mult)
            attn = epool.tile([NW, NB], bf, tag="attn")
            sm = epool.tile([NW, 1], f32, tag="sm")
            nc.scalar.activation(attn[:], sc_ps[:], AF.Exp, bias=mxn[:], scale=1.0 / 32.0,
                                 accum_out=sm[:])
            rs = epool.tile([NW, 1], f32, tag="rs")
            nc.vector.reciprocal(rs[:], sm[:])
            nc.vector.tensor_scalar(attn[:], attn[:], rs[:], None, op0=ALU.mult)
            # attnT
            attnT = epool.tile([128, NBC, NW], bf, tag="attnT")
            ta_f = pt.tile([128, 1024], bf, tag="tr")
            ta = ta_f[:, 0 : NBC * NW]
            for c in range(NBC):
                nc.tensor.transpose(ta[:, c * NW : (c + 1) * NW],
                                    attn[:, c * 128 : (c + 1) * 128], ident[0:NW, 0:NW])
            nc.vector.tensor_copy(attnT[:], ta[:])
            # uT = (attn @ bw)^T directly: [128, dc, 5]
            uT_ps = psm.tile([128, FDC, NW], f32, tag="ps_sm")
            for dc in range(FDC):
                for c in range(NBC):
                    nc.tensor.matmul(uT_ps[:, dc, :],
                                     bw_nat[e][:, c, dc * 128 : (dc + 1) * 128],
                                     attnT[:, c, :], start=(c == 0), stop=(c == NBC - 1))
            uc = epool.tile([128, FDC, NW], bf, tag="uc")
            nc.vector.tensor_copy(uc[:], uT_ps[:])
            nc.vector.tensor_tensor(ugT[:, :, e * NW : (e + 1) * NW], uc[:],
                                    gvis5T[:, :, e * NW : (e + 1) * NW], op=ALU.mult)

        # ---------------- outT = ((u*g) @ Wv)^T ----------------
        outT_ps = psm.tile([128, FDC, B20], f32, tag="ps_sm")
        for m in range(FDC):
            for kc in range(FDC):
                nc.tensor.matmul(outT_ps[:, m, :], wv[:, kc, m * 128 : (m + 1) * 128],
                                 ugT[:, kc, :], start=(kc == 0), stop=(kc == FDC - 1))
        outT = apool.tile([128, FDC, B20], bf, tag="outT")
        nc.vector.tensor_copy(outT[:], outT_ps[:])

        # ---------------- out2 = out @ Wfc + sc ; fake; pnT fake col ----------------
        out2_ps0 = pbig.tile([B20, 512], f32, tag="ps_big")
        out2_ps1 = pbig.tile([B20, 512], f32, tag="ps_big")
        out2_ps = [out2_ps0, out2_ps1]
        out2 = apool.tile([B20, FD], f32, tag="out2")
        for h in range(2):
            for kc in range(FDC):
                nc.tensor.matmul(out2_ps[h][:, :], outT[:, kc, :],
                                 wfc[:, kc, h * 512 : (h + 1) * 512],
                                 start=(kc == 0), stop=(kc == FDC - 1))
            nc.vector.tensor_tensor(out2[:, h * 512 : (h + 1) * 512], out2_ps[h][:, :],
                                    sc_nat[:, h * 512 : (h + 1) * 512], op=ALU.add)
        fk_ps0 = pbig.tile([B20, 512], f32, tag="ps_big")
        fk_ps1 = pbig.tile([B20, 512], f32, tag="ps_big")
        fk_ps = [fk_ps0, fk_ps1]
        for h in range(2):
            nc.tensor.matmul(fk_ps[h][0:EPC, 0:512], fifths[:].bitcast(f32r),
                             out2[:, h * 512 : (h + 1) * 512].bitcast(f32r),
                             start=True, stop=True)
        fk = spool.tile([EPC, FD], f32, tag="fk")
        ssf = spool.tile([EPC, 1], f32, tag="fk_ssq")
        sqf = spool.tile([NQ, FD], bf, tag="scratch4k")
        for h in range(2):
            nc.vector.tensor_copy(fk[:, h * 512 : (h + 1) * 512], fk_ps[h][0:EPC, 0:512])
        nc.scalar.activation(sqf[0:EPC, :], fk[:], AF.Square, accum_out=ssf[:])
        rf = spool.tile([EPC, 1], f32, tag="fk_r")
        nc.vector.reciprocal(rf[:], ssf[:])
        inv_f = spool.tile([EPC, 1], f32, tag="fk_inv")
        nc.scalar.activation(inv_f[:], rf[:], AF.Sqrt)
        pn_fk = spool.tile([EPC, FD], bf, tag="pn_fk")
        nc.vector.tensor_scalar(pn_fk[:], fk[:], inv_f[:], None, op0=ALU.mult)
        tf_f = pt.tile([128, 1024], bf, tag="tr")
        tf = tf_f[:, 0 : FDC * EPC]
        for dc in range(FDC):
            nc.tensor.transpose(tf[:, dc * EPC : (dc + 1) * EPC],
                                pn_fk[:, dc * 128 : (dc + 1) * 128], ident[0:EPC, 0:EPC])
        for dc in range(FDC):
            dst = pnT[:, dc, :].rearrange("p (e s) -> p e s", s=NPROTO)[:, :, NW]
            nc.vector.tensor_copy(dst, tf[:, dc * EPC : (dc + 1) * EPC])

        # ---------------- logits ----------------
        lg_ps = psm.tile([NQ, EPC * NPROTO], f32, tag="ps_sm")
        for e in range(EPC):
            for dc in range(FDC):
                nc.tensor.matmul(lg_ps[:, e * NPROTO : (e + 1) * NPROTO],
                                 qnT_l[e][:, dc, :],
                                 pnT[:, dc, e * NPROTO : (e + 1) * NPROTO],
                                 start=(dc == 0), stop=(dc == FDC - 1))
        lg = spool.tile([NQ, EPC * NPROTO], f32, tag="lg")
        nc.vector.tensor_copy(lg[:], lg_ps[:])
        for e in range(EPC):
            nc.scalar.dma_start(out_d.ap()[e], lg[:, e * NPROTO : (e + 1) * NPROTO])

    nc.finalize()
    return nc


def _prep_shared(inputs):
    """bf16-convert / transpose / pad the replicated weights (host side)."""
    def b(a):
        return np.ascontiguousarray(np.asarray(a).astype(BF16))

    def padr(a, n):
        p = np.zeros((n - a.shape[0],) + a.shape[1:], a.dtype)
        return np.ascontiguousarray(np.concatenate([a, p], axis=0))

    f32 = np.float32
    Wm1 = np.asarray(inputs["Wm1"], f32)
    Wm2 = np.asarray(inputs["Wm2"], f32)
    Wq = np.asarray(inputs["Wq"], f32)
    Wqs = np.asarray(inputs["Wqs"], f32)
    Wk = np.asarray(inputs["Wk"], f32)
    Wks = np.asarray(inputs["Wks"], f32)
    Wv = np.asarray(inputs["Wv"], f32)
    Wfc = np.asarray(inputs["Wfc"], f32)
    Wvis = np.asarray(inputs["Wvis"], f32)
    Wsem = np.asarray(inputs["Wsem"], f32)

    bm1c = np.zeros((128, 3), f32)
    bm2c = np.zeros((128, 3), f32)
    bm1 = np.asarray(inputs["bm1"], f32).reshape(-1)
    bm2 = np.asarray(inputs["bm2"], f32).reshape(-1)
    for c, (off, sz) in enumerate(SEMCH):
        bm1c[0:sz, c] = bm1[off : off + sz]
        bm2c[0:sz, c] = bm2[off : off + sz]

    esel = np.zeros((EPC, 128, B20), f32)
    for e in range(EPC):
        esel[e, :, e * NW : (e + 1) * NW] = 1.0 / NB
    fifths = np.zeros((B20, EPC), f32)
    for e in range(EPC):
        fifths[e * NW : (e + 1) * NW, e] = 1.0 / NW

    ident = np.eye(128, dtype=f32)

    return {
        "Wm1_pad": b(padr(Wm1, SEMP)),
        "Wm2_pad": b(padr(Wm2, SEMP)),
        "Wq": b(Wq),
        "Wqs_pad": b(padr(Wqs, SEMP)),
        "WkT": b(Wk.T),
        "WksT": b(Wks.T),
        "Wv": b(Wv),
        "Wfc": b(Wfc),
        "Wvis_pad": b(padr(Wvis, VINP)),
        "Wsem_pad": b(padr(Wsem, VINP)),
        "bm1c": bm1c,
        "bm2c": bm2c,
        "bvis_row": b(np.asarray(inputs["bvis"], f32).reshape(1, FD)),
        "bsem_row": b(np.asarray(inputs["bsem"], f32).reshape(1, SEM)),
        "esel": b(esel),
        "fifths": fifths,
        "ones20": b(np.ones((1, B20), f32)),
        "ident_bf": b(ident),
    }


def kernel(**inputs):
    from concourse.bass_utils import run_bass_kernel_spmd

    temp = float(np.asarray(inputs["temp"]))
    key = ("v4", temp)
    if key not in _MODULE_CACHE:
        _MODULE_CACHE[key] = _build_module(temp)
    nc = _MODULE_CACHE[key]

    shared = _prep_shared(inputs)
    sc_f = np.asarray(inputs["support_center"], np.float32)
    ss_f = np.asarray(inputs["support_seman"], np.float32)
    bw_f = np.asarray(inputs["base_weights"])
    bsm_f = np.asarray(inputs["base_seman"])
    qf_f = np.asarray(inputs["query_feature"])

    in_maps = []
    for cid in range(NCORES):
        lo, hi = cid * EPC, (cid + 1) * EPC
        sc20 = np.ascontiguousarray(sc_f[lo:hi].reshape(B20, FD))
        ss20 = ss_f[lo:hi].reshape(B20, SEM)
        ssT = np.zeros((3, 128, B20), np.float32)
        for c, (off, sz) in enumerate(SEMCH):
            ssT[c, 0:sz, :] = ss20[:, off : off + sz].T
        m = dict(shared)
        m["sc_nat"] = sc20
        m["scT"] = np.ascontiguousarray(sc20.T.astype(BF16))
        m["ssT"] = np.ascontiguousarray(ssT.astype(BF16))
        m["bw"] = np.ascontiguousarray(bw_f[lo:hi].astype(BF16))
        m["bsm"] = np.ascontiguousarray(bsm_f[lo:hi].astype(BF16))
        m["qf"] = np.ascontiguousarray(qf_f[lo:hi].astype(BF16))
        in_maps.append(m)

    res = run_bass_kernel_spmd(nc, in_maps, core_ids=list(range(NCORES)))
    out = np.concatenate([res.results[c]["out"] for c in range(NCORES)], axis=0)
    return out.astype(np.float32)


# revision 9
# speedup vs baseline: 1.9419x; 1.0471x over previous
"""Trainium2 Bass kernel for nn_Classifier_22625887715977 (sparse_attention).

kernel(**inputs) takes FULL unsharded inputs (bs=32) and returns the full
[32, 75, 6] logits. Shards the batch over 8 NeuronCores (4 episodes per
core); weights replicated, converted to bf16 on host, streamed.

Math (per episode, exact reassociation of the reference — never materializes
the expanded per-(episode,way) base bank):
  s      = leaky(ss @ Wm1 + bm1) @ Wm2 + bm2
  avg    = mean_n [bw | bsm]
  gvis   = sigmoid(avg @ Wvis + bvis) + 1 ; gsem = sigmoid(avg @ Wsem + bsem) + 1
  q      = sc @ Wq + s @ Wqs
  scores = ((q @ Wk^T) * gvis) @ bw^T + ((q @ Wks^T) * gsem) @ bsm^T ; attn = softmax(scores/32)
  out    = ((attn @ bw) * gvis) @ Wv ; out2 = out @ Wfc + sc
  fake   = mean_w out2 ; protos = [sc; fake] ; logits = temp * cos(qf, protos)

v5 implementation notes:
 - All large streams (weights, banks, qf) are bf16 (host-converted); matmuls
   run bf16 x bf16 -> f32 PSUM. The residual / fake-proto path stays f32.
 - Wk, Wks, sc, ss are host-TRANSPOSED; projection outputs are computed
   directly in transposed form (qT, t1T, t2T, uT, outT) with tiny free sizes.
 - Gates are computed way-replicated ([B20, .] via an episode-selector lhsT)
   so each gate application is a single elementwise multiply.
 - DMA in global need-order: weights on sync; banks split across gpsimd and
   scalar queues so they land early; Wv / Wfc (tail deps) stream last.
 - Per-episode attention runs stage-parallel (scores x4 pipelined through
   softmax / attnT / uT) to hide cross-engine latency.
"""

import numpy as np
import ml_dtypes

BF16 = ml_dtypes.bfloat16

BS = 32
NCORES = 8
EPC = BS // NCORES          # 4 episodes per core
NW = 5
B20 = EPC * NW              # 20
FD = 1024
FDC = FD // 128             # 8
SEM = 300
SEMCH = [(0, 128), (128, 128), (256, 44)]
SEMP = 384                  # SEM padded to 3*128
NB = 512
NBC = NB // 128             # 4
NQ = 75
NPROTO = NW + 1             # 6
VINC = 11                   # ceil(1324/128)
VINP = VINC * 128           # 1408

_MODULE_CACHE = {}


def _build_module(temp: float):
    import concourse.mybir as mybir
    import concourse.tile as tile
    from concourse import bacc

    f32 = mybir.dt.float32
    f32r = mybir.dt.float32r
    bf = mybir.dt.bfloat16
    AF = mybir.ActivationFunctionType
    ALU = mybir.AluOpType
    AX = mybir.AxisListType

    nc = bacc.Bacc("TRN2", target_bir_lowering=False, debug=False)

    def di(name, shape, dt=f32):
        return nc.dram_tensor(name, shape, dt, kind="ExternalInput")

    sc_d = di("sc_nat", [B20, FD])
    fifths_d = di("fifths", [B20, EPC])
    bm1_d = di("bm1c", [128, 3])
    bm2_d = di("bm2c", [128, 3])
    scT_d = di("scT", [FD, B20], bf)
    ssT_d = di("ssT", [3, 128, B20], bf)
    esel_d = di("esel", [EPC, 128, B20], bf)
    ones_d = di("ones20", [1, B20], bf)
    bvis_d = di("bvis_row", [1, FD], bf)
    bsem_d = di("bsem_row", [1, SEM], bf)
    ident_d = di("ident_bf", [128, 128], bf)
    wm1_d = di("Wm1_pad", [SEMP, SEM], bf)
    wm2_d = di("Wm2_pad", [SEMP, SEM], bf)
    wq_d = di("Wq", [FD, FD], bf)
    wqs_d = di("Wqs_pad", [SEMP, FD], bf)
    wkT_d = di("WkT", [FD, FD], bf)
    wksT_d = di("WksT", [FD, SEM], bf)
    wv_d = di("Wv", [FD, FD], bf)
    wfc_d = di("Wfc", [FD, FD], bf)
    wvis_d = di("Wvis_pad", [VINP, FD], bf)
    wsem_d = di("Wsem_pad", [VINP, SEM], bf)
    bw_d = di("bw", [EPC, NB, FD], bf)
    bsm_d = di("bsm", [EPC, NB, SEM], bf)
    qf_d = di("qf", [EPC, NQ, FD], bf)
    out_d = nc.dram_tensor("out", [EPC, NQ, NPROTO], f32, kind="ExternalOutput")

    from contextlib import ExitStack
    with tile.TileContext(nc) as tc, ExitStack() as _ctx:
        def _pool(**kw):
            return _ctx.enter_context(tc.tile_pool(**kw))

        cpool = _pool(name="const", bufs=1)
        wres = _pool(name="wres", bufs=1)
        wbig = _pool(name="wbig", bufs=2)       # Wq,WkT then Wv,Wfc reuse
        wvp = _pool(name="wvis", bufs=1)
        bpool = _pool(name="banks", bufs=1)     # per-e tags
        tpool = _pool(name="bankT", bufs=1)     # per-e tags
        apool = _pool(name="acts", bufs=1)
        spool = _pool(name="small", bufs=1)
        ep2 = _pool(name="ep2", bufs=2)         # qf-normalize pipeline
        ep4 = _pool(name="ep4", bufs=4)         # attention stage pipeline
        qpool = _pool(name="qn", bufs=1)
        pt = _pool(name="pt", bufs=2, space="PSUM")
        pbig = _pool(name="pbig", bufs=2, space="PSUM")
        psm = _pool(name="psm", bufs=2, space="PSUM")
        pscore = _pool(name="pscore", bufs=2, space="PSUM")

        # ---------------- DMA issue ----------------
        # scalar queue: smalls, then odd-episode banks, then qf (cycling)
        ident = cpool.tile([128, 128], bf, tag="ident")
        nc.scalar.dma_start(ident[:], ident_d.ap())
        scT = cpool.tile([128, FDC, B20], bf, tag="scT")
        nc.scalar.dma_start(scT[:], scT_d.ap().rearrange("(c p) b -> p c b", p=128))
        ssT = cpool.tile([128, 3, B20], bf, tag="ssT")
        nc.scalar.dma_start(ssT[:], ssT_d.ap().rearrange("c p b -> p c b"))
        sc_nat = cpool.tile([B20, FD], f32, tag="sc_nat")
        nc.scalar.dma_start(sc_nat[:], sc_d.ap())
        esel = cpool.tile([128, EPC, B20], bf, tag="esel")
        nc.scalar.dma_start(esel[:], esel_d.ap().rearrange("e p b -> p e b"))
        fifths = cpool.tile([B20, EPC], f32, tag="fifths")
        nc.scalar.dma_start(fifths[:], fifths_d.ap())
        ones20 = cpool.tile([1, B20], bf, tag="ones20")
        nc.scalar.dma_start(ones20[:], ones_d.ap())
        bm1c = cpool.tile([128, 3], f32, tag="bm1c")
        nc.scalar.dma_start(bm1c[:], bm1_d.ap())
        bm2c = cpool.tile([128, 3], f32, tag="bm2c")
        nc.scalar.dma_start(bm2c[:], bm2_d.ap())
        bvis_row = cpool.tile([1, FD], bf, tag="bvis")
        nc.scalar.dma_start(bvis_row[:], bvis_d.ap())
        bsem_row = cpool.tile([1, SEM], bf, tag="bsem")
        nc.scalar.dma_start(bsem_row[:], bsem_d.ap())
        wm1 = wres.tile([128, 3, SEM], bf, tag="wm1")
        nc.scalar.dma_start(wm1[:], wm1_d.ap().rearrange("(c p) d -> p c d", p=128))
        wm2 = wres.tile([128, 3, SEM], bf, tag="wm2")
        nc.scalar.dma_start(wm2[:], wm2_d.ap().rearrange("(c p) d -> p c d", p=128))

        bw_nat, bsm_nat = [None] * EPC, [None] * EPC
        for e in range(EPC):
            q = nc.gpsimd if e % 2 == 0 else nc.scalar
            bwt = bpool.tile([128, NBC, FD], bf, tag=f"bw{e}")
            q.dma_start(bwt[:], bw_d.ap()[e].rearrange("(c p) d -> p c d", p=128))
            bw_nat[e] = bwt
            bst = bpool.tile([128, NBC, SEM], bf, tag=f"bsm{e}")
            q.dma_start(bst[:], bsm_d.ap()[e].rearrange("(c p) d -> p c d", p=128))
            bsm_nat[e] = bst

        # sync queue: big weights in need-order
        wq = wbig.tile([128, FDC, FD], bf, tag="wbig")
        nc.sync.dma_start(wq[:], wq_d.ap().rearrange("(c p) d -> p c d", p=128))
        wqs = wres.tile([128, 3, FD], bf, tag="wqs")
        nc.sync.dma_start(wqs[:], wqs_d.ap().rearrange("(c p) d -> p c d", p=128))
        wkT = wbig.tile([128, FDC, FD], bf, tag="wbig")
        nc.sync.dma_start(wkT[:], wkT_d.ap().rearrange("(c p) d -> p c d", p=128))
        wksT = wres.tile([128, FDC, SEM], bf, tag="wksT")
        nc.sync.dma_start(wksT[:], wksT_d.ap().rearrange("(c p) d -> p c d", p=128))
        wvis = wvp.tile([128, VINC, FD], bf, tag="wvis")
        nc.sync.dma_start(wvis[:], wvis_d.ap().rearrange("(c p) d -> p c d", p=128))
        wsem = wvp.tile([128, VINC, SEM], bf, tag="wsem")
        nc.sync.dma_start(wsem[:], wsem_d.ap().rearrange("(c p) d -> p c d", p=128))
        wv = wbig.tile([128, FDC, FD], bf, tag="wbig")
        nc.sync.dma_start(wv[:], wv_d.ap().rearrange("(c p) d -> p c d", p=128))
        wfc = wbig.tile([128, FDC, FD], bf, tag="wbig")
        for h in range(2):
            nc.sync.dma_start(
                wfc[:, :, h * 512 : (h + 1) * 512],
                wfc_d.ap()[:, h * 512 : (h + 1) * 512]
                .rearrange("(c p) d -> p c d", p=128))

        # memset partial-chunk tiles that are read at full 128 partitions
        h1T = apool.tile([128, 3, B20], bf, tag="h1T")
        sT = apool.tile([128, 3, B20], bf, tag="sT")
        avgsT = apool.tile([128, 3, B20], bf, tag="avgsT")
        nc.vector.memset(h1T[:], 0.0)
        nc.vector.memset(sT[:], 0.0)
        nc.vector.memset(avgsT[:], 0.0)

        _eng = [nc.vector, nc.scalar, nc.gpsimd]
        _ei = [0]

        def copy_rr(dst, src):
            e = _eng[_ei[0] % 3]
            _ei[0] += 1
            if e is nc.scalar:
                e.copy(dst, src)
            else:
                e.tensor_copy(dst, src)

        # ---------------- sMLP ----------------
        for mc, (moff, msz) in enumerate(SEMCH):
            ph = psm.tile([128, B20], f32, tag="ps_sm")
            for kc, (koff, ksz) in enumerate(SEMCH):
                nc.tensor.matmul(ph[0:msz, :], wm1[0:ksz, kc, moff : moff + msz],
                                 ssT[0:ksz, kc, :], start=(kc == 0), stop=(kc == 2))
            lk = spool.tile([128, B20], f32, tag="mlp_lk")
            nc.vector.tensor_scalar(lk[0:msz, :], ph[0:msz, :], bm1c[0:msz, mc : mc + 1],
                                    0.1, op0=ALU.add, op1=ALU.mult)
            nc.vector.tensor_scalar(h1T[0:msz, mc, :], ph[0:msz, :],
                                    bm1c[0:msz, mc : mc + 1], None, op0=ALU.add)
            nc.vector.tensor_tensor(h1T[0:msz, mc, :], h1T[0:msz, mc, :], lk[0:msz, :],
                                    op=ALU.max)
        for mc, (moff, msz) in enumerate(SEMCH):
            ph = psm.tile([128, B20], f32, tag="ps_sm")
            for kc, (koff, ksz) in enumerate(SEMCH):
                nc.tensor.matmul(ph[0:msz, :], wm2[0:ksz, kc, moff : moff + msz],
                                 h1T[0:ksz, kc, :], start=(kc == 0), stop=(kc == 2))
            nc.vector.tensor_scalar(sT[0:msz, mc, :], ph[0:msz, :],
                                    bm2c[0:msz, mc : mc + 1], None, op0=ALU.add)

        # ---------------- pn_sc = sc/||sc|| (early) + pnT sc columns ----------------
        ssq_sc = spool.tile([B20, 1], f32, tag="sc_ssq")
        sq_sc = spool.tile([NQ, FD], bf, tag="scratch4k")
        nc.scalar.activation(sq_sc[0:B20, :], sc_nat[:], AF.Square, accum_out=ssq_sc[:])
        r_sc = spool.tile([B20, 1], f32, tag="sc_r")
        nc.vector.reciprocal(r_sc[:], ssq_sc[:])
        inv_sc = spool.tile([B20, 1], f32, tag="sc_inv")
        nc.scalar.activation(inv_sc[:], r_sc[:], AF.Sqrt)
        pn_sc = spool.tile([B20, FD], bf, tag="pn_sc")
        nc.vector.tensor_scalar(pn_sc[:], sc_nat[:], inv_sc[:], None, op0=ALU.mult)
        pnT = apool.tile([128, FDC, EPC * NPROTO], bf, tag="pnT")
        for g in range(2):
            tfull = pt.tile([128, 1024], bf, tag="tr")
            t = tfull[:, 0 : 4 * B20]
            for i in range(4):
                dc = g * 4 + i
                nc.tensor.transpose(t[:, i * B20 : (i + 1) * B20],
                                    pn_sc[:, dc * 128 : (dc + 1) * 128], ident[0:B20, 0:B20])
            for i in range(4):
                dc = g * 4 + i
                dst = pnT[:, dc, :].rearrange("p (e s) -> p e s", s=NPROTO)[:, :, 0:NW]
                src = t[:, i * B20 : (i + 1) * B20].rearrange("p (e w) -> p e w", w=NW)
                nc.vector.tensor_copy(dst, src)

        # ---------------- qT = (sc@Wq + s@Wqs)^T directly ----------------
        qT_ps = psm.tile([128, FDC, B20], f32, tag="ps_sm")
        for m in range(FDC):
            for kc in range(FDC):
                nc.tensor.matmul(qT_ps[:, m, :], wq[:, kc, m * 128 : (m + 1) * 128],
                                 scT[:, kc, :], start=(kc == 0), stop=False)
            for c in range(3):
                nc.tensor.matmul(qT_ps[:, m, :], wqs[:, c, m * 128 : (m + 1) * 128],
                                 sT[:, c, :], start=False, stop=(c == 2))
        qT = apool.tile([128, FDC, B20], bf, tag="qT")
        nc.vector.tensor_copy(qT[:], qT_ps[:])

        # ---------------- per-episode banks: avg chunks + transposes ----------------
        avgv_ps0 = pbig.tile([B20, 512], f32, tag="ps_big")
        avgv_ps1 = pbig.tile([B20, 512], f32, tag="ps_big")
        avgv_ps = [avgv_ps0, avgv_ps1]
        avgs_ps = psm.tile([B20, SEM], f32, tag="ps_sm")
        bwT_l, bsmT_l = [], []
        t1_done = False
        t1c = apool.tile([128, FDC, B20], bf, tag="t1c")
        t2c = apool.tile([128, 3, B20], bf, tag="t2c")
        for e in range(EPC):
            for c in range(NBC):
                for h in range(2):
                    nc.tensor.matmul(avgv_ps[h][:, :], esel[:, e, :],
                                     bw_nat[e][:, c, h * 512 : (h + 1) * 512],
                                     start=(e == 0 and c == 0),
                                     stop=(e == EPC - 1 and c == NBC - 1))
            bwT = tpool.tile([128, FDC, NB], bf, tag=f"bwT{e}")
            for g in range(4):
                t = pt.tile([128, 1024], bf, tag="tr")
                for i in range(2):
                    dc = g * 2 + i
                    for c in range(NBC):
                        nc.tensor.transpose(
                            t[:, i * 512 + c * 128 : i * 512 + (c + 1) * 128],
                            bw_nat[e][:, c, dc * 128 : (dc + 1) * 128],
                            ident[:])
                copy_rr(bwT[:, g * 2 : g * 2 + 2, :], t[:])
            bwT_l.append(bwT)
            for c in range(NBC):
                nc.tensor.matmul(avgs_ps[:, :], esel[:, e, :], bsm_nat[e][:, c, :],
                                 start=(e == 0 and c == 0),
                                 stop=(e == EPC - 1 and c == NBC - 1))
            bsmT = tpool.tile([128, 3, NB], bf, tag=f"bsmT{e}")
            t2p = pt.tile([128, 1024], bf, tag="tr")
            for sci, (soff, ssz) in enumerate(SEMCH[:2]):
                for c in range(NBC):
                    nc.tensor.transpose(
                        t2p[:, sci * 512 + c * 128 : sci * 512 + (c + 1) * 128],
                        bsm_nat[e][:, c, soff : soff + ssz], ident[:])
            copy_rr(bsmT[:, 0:2, :], t2p[:])
            t3p = pt.tile([128, 1024], bf, tag="tr")
            soff, ssz = SEMCH[2]
            for c in range(NBC):
                nc.tensor.transpose(t3p[0:ssz, c * 128 : (c + 1) * 128],
                                    bsm_nat[e][:, c, soff : soff + ssz],
                                    ident[:])
            copy_rr(bsmT[0:ssz, 2, :], t3p[0:ssz, 0:512])
            bsmT_l.append(bsmT)

            if not t1_done:
                # t1T/t2T after first bank (WkT arrives around now)
                t1_done = True
                t1_ps = psm.tile([128, FDC, B20], f32, tag="ps_sm")
                for m in range(FDC):
                    for kc in range(FDC):
                        nc.tensor.matmul(t1_ps[:, m, :],
                                         wkT[:, kc, m * 128 : (m + 1) * 128],
                                         qT[:, kc, :], start=(kc == 0),
                                         stop=(kc == FDC - 1))
                nc.vector.tensor_copy(t1c[:], t1_ps[:])
                t2_ps = psm.tile([128, 3, B20], f32, tag="ps_sm")
                for mc, (moff, msz) in enumerate(SEMCH):
                    for kc in range(FDC):
                        nc.tensor.matmul(t2_ps[0:msz, mc, :],
                                         wksT[:, kc, moff : moff + msz],
                                         qT[:, kc, :], start=(kc == 0),
                                         stop=(kc == FDC - 1))
                nc.vector.tensor_copy(t2c[:], t2_ps[:])

        # ---------------- avg transposes -> way-replicated avgT ----------------
        avgv_nat = spool.tile([B20, FD], bf, tag="avgv_nat")
        for h in range(2):
            nc.vector.tensor_copy(avgv_nat[:, h * 512 : (h + 1) * 512], avgv_ps[h][:, :])
        avgs_nat = spool.tile([B20, SEM], bf, tag="avgs_nat")
        nc.vector.tensor_copy(avgs_nat[:], avgs_ps[:])
        avgvT = apool.tile([128, FDC, B20], bf, tag="avgvT")
        for g in range(2):
            tfull = pt.tile([128, 1024], bf, tag="tr")
            t = tfull[:, 0 : 4 * B20]
            for i in range(4):
                dc = g * 4 + i
                nc.tensor.transpose(t[:, i * B20 : (i + 1) * B20],
                                    avgv_nat[:, dc * 128 : (dc + 1) * 128],
                                    ident[0:B20, 0:B20])
            copy_rr(avgvT[:, g * 4 : (g + 1) * 4, :], t[:])
        tsp_f = pt.tile([128, 1024], bf, tag="tr")
        tsp = tsp_f[:, 0 : 3 * B20]
        for sci, (soff, ssz) in enumerate(SEMCH):
            nc.tensor.transpose(tsp[0:ssz, sci * B20 : (sci + 1) * B20],
                                avgs_nat[:, soff : soff + ssz], ident[0:B20, 0:B20])
        nc.vector.tensor_copy(avgsT[0:128, 0, :], tsp[0:128, 0:B20])
        nc.vector.tensor_copy(avgsT[0:128, 1, :], tsp[0:128, B20 : 2 * B20])
        soff, ssz = SEMCH[2]
        nc.vector.tensor_copy(avgsT[0:ssz, 2, :], tsp[0:ssz, 2 * B20 : 3 * B20])

        # ---------------- gates ----------------
        gpv_ps0 = pbig.tile([B20, 512], f32, tag="ps_big")
        gpv_ps1 = pbig.tile([B20, 512], f32, tag="ps_big")
        gpv_ps = [gpv_ps0, gpv_ps1]
        for h in range(2):
            for kc in range(FDC):
                nc.tensor.matmul(gpv_ps[h][:, :], avgvT[:, kc, :],
                                 wvis[:, kc, h * 512 : (h + 1) * 512],
                                 start=(kc == 0), stop=False)
            for c in range(3):
                nc.tensor.matmul(gpv_ps[h][:, :], avgsT[:, c, :],
                                 wvis[:, FDC + c, h * 512 : (h + 1) * 512],
                                 start=False, stop=False)
            nc.tensor.matmul(gpv_ps[h][:, :], ones20[:], bvis_row[:, h * 512 : (h + 1) * 512],
                             start=False, stop=True)
        gps_ps = psm.tile([B20, SEM], f32, tag="ps_sm")
        for kc in range(FDC):
            nc.tensor.matmul(gps_ps[:, :], avgvT[:, kc, :], wsem[:, kc, :],
                             start=(kc == 0), stop=False)
        for c in range(3):
            nc.tensor.matmul(gps_ps[:, :], avgsT[:, c, :], wsem[:, FDC + c, :],
                             start=False, stop=False)
        nc.tensor.matmul(gps_ps[:, :], ones20[:], bsem_row[:], start=False, stop=True)

        gv_nat = spool.tile([B20, FD], bf, tag="avgv_nat")
        for h in range(2):
            nc.scalar.activation(gv_nat[:, h * 512 : (h + 1) * 512], gpv_ps[h][:, :],
                                 AF.Sigmoid)
        gs_nat = spool.tile([B20, SEM], bf, tag="avgs_nat")
        nc.scalar.activation(gs_nat[:], gps_ps[:], AF.Sigmoid)

        gvis5T = apool.tile([128, FDC, B20], bf, tag="gvis5T")
        for g in range(2):
            tfull = pt.tile([128, 1024], bf, tag="tr")
            t = tfull[:, 0 : 4 * B20]
            for i in range(4):
                dc = g * 4 + i
                nc.tensor.transpose(t[:, i * B20 : (i + 1) * B20],
                                    gv_nat[:, dc * 128 : (dc + 1) * 128],
                                    ident[0:B20, 0:B20])
            nc.vector.tensor_scalar_add(gvis5T[:, g * 4 : (g + 1) * 4, :], t[:], 1.0)
        gsem5T = apool.tile([128, 3, B20], bf, tag="gsem5T")
        tg_f = pt.tile([128, 1024], bf, tag="tr")
        tg = tg_f[:, 0 : 3 * B20]
        for sci, (soff, ssz) in enumerate(SEMCH):
            nc.tensor.transpose(tg[0:ssz, sci * B20 : (sci + 1) * B20],
                                gs_nat[:, soff : soff + ssz], ident[0:B20, 0:B20])
        nc.vector.tensor_scalar_add(gsem5T[0:128, 0:2, :],
                                    tg[0:128, 0 : 2 * B20].rearrange("p (c b) -> p c b", c=2),
                                    1.0)
        soff, ssz = SEMCH[2]
        nc.vector.tensor_scalar_add(gsem5T[0:ssz, 2, :], tg[0:ssz, 2 * B20 : 3 * B20], 1.0)

        t1gT = apool.tile([128, FDC, B20], bf, tag="t1gT")
        nc.vector.tensor_tensor(t1gT[:], t1c[:], gvis5T[:], op=ALU.mult)
        t2gT = apool.tile([128, 3, B20], bf, tag="t2gT")
        nc.vector.tensor_tensor(t2gT[:], t2c[:], gsem5T[:], op=ALU.mult)

        # ---------------- attention: stage-parallel over episodes ----------------
        def scores_mm(e):
            sc_ps = pscore.tile([NW, NB], f32, tag="ps_sc")
            for dc in range(FDC):
                nc.tensor.matmul(sc_ps[:], t1gT[:, dc, e * NW : (e + 1) * NW],
                                 bwT_l[e][:, dc, :], start=(dc == 0), stop=False)
            for sci, (soff, ssz) in enumerate(SEMCH):
                nc.tensor.matmul(sc_ps[:], t2gT[0:ssz, sci, e * NW : (e + 1) * NW],
                                 bsmT_l[e][0:ssz, sci, :], start=False, stop=(sci == 2))
            return sc_ps

        def softmax(e, sc_ps):
            mx = ep4.tile([NW, 1], f32, tag="mx")
            nc.vector.reduce_max(mx[:], sc_ps[:], axis=AX.X)
            mxn = ep4.tile([NW, 1], f32, tag="mxn")
            nc.vector.tensor_scalar(mxn[:], mx[:], -1.0 / 32.0, None, op0=ALU.mult)
            attn = ep4.tile([NW, NB], bf, tag="attn")
            sm = ep4.tile([NW, 1], f32, tag="sm")
            nc.scalar.activation(attn[:], sc_ps[:], AF.Exp, bias=mxn[:], scale=1.0 / 32.0,
                                 accum_out=sm[:])
            rs = ep4.tile([NW, 1], f32, tag="rs")
            nc.vector.reciprocal(rs[:], sm[:])
            nc.vector.tensor_scalar(attn[:], attn[:], rs[:], None, op0=ALU.mult)
            return attn

        def attnT_mm(e, attn):
            attnT = ep4.tile([128, NBC, NW], bf, tag="attnT")
            ta_f = pt.tile([128, 1024], bf, tag="tr")
            ta = ta_f[:, 0 : NBC * NW]
            for c in range(NBC):
                nc.tensor.transpose(ta[:, c * NW : (c + 1) * NW],
                                    attn[:, c * 128 : (c + 1) * 128], ident[0:NW, 0:NW])
            nc.vector.tensor_copy(attnT[:], ta[:])
            return attnT

        ugT = apool.tile([128, FDC, B20], bf, tag="ugT")

        def uT_mm(e, attnT):
            uT_ps = psm.tile([128, FDC, NW], f32, tag="ps_sm")
            for dc in range(FDC):
                for c in range(NBC):
                    nc.tensor.matmul(uT_ps[:, dc, :],
                                     bw_nat[e][:, c, dc * 128 : (dc + 1) * 128],
                                     attnT[:, c, :], start=(c == 0), stop=(c == NBC - 1))
            nc.vector.tensor_tensor(ugT[:, :, e * NW : (e + 1) * NW], uT_ps[:],
                                    gvis5T[:, :, e * NW : (e + 1) * NW], op=ALU.mult)

        sc_ps_l = [None] * EPC
        attn_l = [None] * EPC
        attnT_list = [None] * EPC
        sc_ps_l[0] = scores_mm(0)
        sc_ps_l[1] = scores_mm(1)
        attn_l[0] = softmax(0, sc_ps_l[0])
        attnT_list[0] = attnT_mm(0, attn_l[0])
        sc_ps_l[2] = scores_mm(2)
        attn_l[1] = softmax(1, sc_ps_l[1])
        uT_mm(0, attnT_list[0])
        attnT_list[1] = attnT_mm(1, attn_l[1])
        sc_ps_l[3] = scores_mm(3)
        attn_l[2] = softmax(2, sc_ps_l[2])
        uT_mm(1, attnT_list[1])
        attnT_list[2] = attnT_mm(2, attn_l[2])
        attn_l[3] = softmax(3, sc_ps_l[3])
        uT_mm(2, attnT_list[2])
        attnT_list[3] = attnT_mm(3, attn_l[3])
        uT_mm(3, attnT_list[3])

        # ---------------- qf normalize + qnT (banks done; qf arrives now) -------
        qnT_l = []
        for e in range(EPC):
            qf_e = ep2.tile([NQ, FD], bf, tag="qf")
            nc.scalar.dma_start(qf_e[:], qf_d.ap()[e])
            ssq = ep2.tile([NQ, 1], f32, tag="q_ssq")
            sq = spool.tile([NQ, FD], bf, tag="scratch4k")
            nc.scalar.activation(sq[:], qf_e[:], AF.Square, accum_out=ssq[:])
            rq = ep2.tile([NQ, 1], f32, tag="q_rq")
            nc.vector.reciprocal(rq[:], ssq[:])
            s10 = ep2.tile([NQ, 1], f32, tag="q_s10")
            nc.scalar.activation(s10[:], rq[:], AF.Sqrt, scale=float(temp) * float(temp))
            qn = ep2.tile([NQ, FD], bf, tag="q_qn")
            nc.vector.tensor_scalar(qn[:], qf_e[:], s10[:], None, op0=ALU.mult)
            qnT = qpool.tile([128, FDC, NQ], bf, tag=f"qnT{e}")
            for g in range(2):
                tfull = pt.tile([128, 1024], bf, tag="tr")
                t = tfull[:, 0 : 4 * NQ]
                for i in range(4):
                    dc = g * 4 + i
                    nc.tensor.transpose(t[:, i * NQ : (i + 1) * NQ],
                                        qn[:, dc * 128 : (dc + 1) * 128], ident[0:NQ, 0:NQ])
                copy_rr(qnT[:, g * 4 : (g + 1) * 4, :], t[:])
            qnT_l.append(qnT)

        # ---------------- outT = ((u*g) @ Wv)^T ----------------
        outT_ps = psm.tile([128, FDC, B20], f32, tag="ps_sm")
        for m in range(FDC):
            for kc in range(FDC):
                nc.tensor.matmul(outT_ps[:, m, :], wv[:, kc, m * 128 : (m + 1) * 128],
                                 ugT[:, kc, :], start=(kc == 0), stop=(kc == FDC - 1))
        outT = apool.tile([128, FDC, B20], bf, tag="outT")
        nc.vector.tensor_copy(outT[:], outT_ps[:])

        # ---------------- out2 = out@Wfc + sc ; fake (per half) ----------------
        out2 = apool.tile([B20, FD], f32, tag="out2")
        fk = spool.tile([EPC, FD], f32, tag="fk")
        ssf_h0 = spool.tile([EPC, 1], f32, tag="fk_ssq0")
        ssf_h1 = spool.tile([EPC, 1], f32, tag="fk_ssq1")
        ssf_h = [ssf_h0, ssf_h1]
        sqf = spool.tile([NQ, FD], bf, tag="scratch4k")
        for h in range(2):
            o2_ps = pbig.tile([B20, 512], f32, tag="ps_big")
            for kc in range(FDC):
                nc.tensor.matmul(o2_ps[:, :], outT[:, kc, :],
                                 wfc[:, kc, h * 512 : (h + 1) * 512],
                                 start=(kc == 0), stop=(kc == FDC - 1))
            nc.vector.tensor_tensor(out2[:, h * 512 : (h + 1) * 512], o2_ps[:, :],
                                    sc_nat[:, h * 512 : (h + 1) * 512], op=ALU.add)
            fk_ps = pbig.tile([B20, 512], f32, tag="ps_big")
            nc.tensor.matmul(fk_ps[0:EPC, 0:512], fifths[:].bitcast(f32r),
                             out2[:, h * 512 : (h + 1) * 512].bitcast(f32r),
                             start=True, stop=True)
            nc.vector.tensor_copy(fk[:, h * 512 : (h + 1) * 512], fk_ps[0:EPC, 0:512])
            nc.scalar.activation(sqf[h * EPC : (h + 1) * EPC, 0:512],
                                 fk[:, h * 512 : (h + 1) * 512], AF.Square,
                                 accum_out=ssf_h[h][:])
        ssf = spool.tile([EPC, 1], f32, tag="fk_ssq")
        nc.vector.tensor_tensor(ssf[:], ssf_h0[:], ssf_h1[:], op=ALU.add)
        rf = spool.tile([EPC, 1], f32, tag="fk_r")
        nc.vector.reciprocal(rf[:], ssf[:])
        inv_f = spool.tile([EPC, 1], f32, tag="fk_inv")
        nc.scalar.activation(inv_f[:], rf[:], AF.Sqrt)
        pn_fk = spool.tile([EPC, FD], bf, tag="pn_fk")
        nc.vector.tensor_scalar(pn_fk[:], fk[:], inv_f[:], None, op0=ALU.mult)
        tf_f = pt.tile([128, 1024], bf, tag="tr")
        tf = tf_f[:, 0 : FDC * EPC]
        for dc in range(FDC):
            nc.tensor.transpose(tf[:, dc * EPC : (dc + 1) * EPC],
                                pn_fk[:, dc * 128 : (dc + 1) * 128], ident[0:EPC, 0:EPC])
        dst = pnT[:].rearrange("p c (e s) -> p c e s", s=NPROTO)[:, :, :, NW]
        nc.vector.tensor_copy(dst, tf[:].rearrange("p (c e) -> p c e", e=EPC))

        # ---------------- logits ----------------
        lg_ps = psm.tile([NQ, EPC * NPROTO], f32, tag="ps_sm")
        for e in range(EPC):
            for dc in range(FDC):
                nc.tensor.matmul(lg_ps[:, e * NPROTO : (e + 1) * NPROTO],
                                 qnT_l[e][:, dc, :],
                                 pnT[:, dc, e * NPROTO : (e + 1) * NPROTO],
                                 start=(dc == 0), stop=(dc == FDC - 1))
        lg = spool.tile([NQ, EPC * NPROTO], f32, tag="lg")
        nc.vector.tensor_copy(lg[:], lg_ps[:])
        for e in range(EPC):
            nc.scalar.dma_start(out_d.ap()[e], lg[:, e * NPROTO : (e + 1) * NPROTO])

    nc.finalize()
    return nc


def _prep_shared(inputs):
    """bf16-convert / transpose / pad the replicated weights (host side)."""
    def b(a):
        return np.ascontiguousarray(np.asarray(a).astype(BF16))

    def padr(a, n):
        p = np.zeros((n - a.shape[0],) + a.shape[1:], a.dtype)
        return np.ascontiguousarray(np.concatenate([a, p], axis=0))

    f32 = np.float32
    bm1c = np.zeros((128, 3), f32)
    bm2c = np.zeros((128, 3), f32)
    bm1 = np.asarray(inputs["bm1"], f32).reshape(-1)
    bm2 = np.asarray(inputs["bm2"], f32).reshape(-1)
    for c, (off, sz) in enumerate(SEMCH):
        bm1c[0:sz, c] = bm1[off : off + sz]
        bm2c[0:sz, c] = bm2[off : off + sz]

    esel = np.zeros((EPC, 128, B20), f32)
    for e in range(EPC):
        esel[e, :, e * NW : (e + 1) * NW] = 1.0 / NB
    fifths = np.zeros((B20, EPC), f32)
    for e in range(EPC):
        fifths[e * NW : (e + 1) * NW, e] = 1.0 / NW

    return {
        "Wm1_pad": b(padr(np.asarray(inputs["Wm1"], f32), SEMP)),
        "Wm2_pad": b(padr(np.asarray(inputs["Wm2"], f32), SEMP)),
        "Wq": b(inputs["Wq"]),
        "Wqs_pad": b(padr(np.asarray(inputs["Wqs"], f32), SEMP)),
        "WkT": b(np.asarray(inputs["Wk"], f32).T),
        "WksT": b(np.asarray(inputs["Wks"], f32).T),
        "Wv": b(inputs["Wv"]),
        "Wfc": b(inputs["Wfc"]),
        "Wvis_pad": b(padr(np.asarray(inputs["Wvis"], f32), VINP)),
        "Wsem_pad": b(padr(np.asarray(inputs["Wsem"], f32), VINP)),
        "bm1c": bm1c,
        "bm2c": bm2c,
        "bvis_row": b(np.asarray(inputs["bvis"], f32).reshape(1, FD)),
        "bsem_row": b(np.asarray(inputs["bsem"], f32).reshape(1, SEM)),
        "esel": b(esel),
        "fifths": fifths,
        "ones20": b(np.ones((1, B20), f32)),
        "ident_bf": b(np.eye(128, dtype=f32)),
    }


def kernel(**inputs):
    from concourse.bass_utils import run_bass_kernel_spmd

    temp = float(np.asarray(inputs["temp"]))
    key = ("v5", temp)
    if key not in _MODULE_CACHE:
        _MODULE_CACHE[key] = _build_module(temp)
    nc = _MODULE_CACHE[key]

    shared = _prep_shared(inputs)
    sc_f = np.asarray(inputs["support_center"], np.float32)
    ss_f = np.asarray(inputs["support_seman"], np.float32)
    bw_f = np.asarray(inputs["base_weights"])
    bsm_f = np.asarray(inputs["base_seman"])
    qf_f = np.asarray(inputs["query_feature"])

    in_maps = []
    for cid in range(NCORES):
        lo, hi = cid * EPC, (cid + 1) * EPC
        sc20 = np.ascontiguousarray(sc_f[lo:hi].reshape(B20, FD))
        ss20 = ss_f[lo:hi].reshape(B20, SEM)
        ssT = np.zeros((3, 128, B20), np.float32)
        for c, (off, sz) in enumerate(SEMCH):
            ssT[c, 0:sz, :] = ss20[:, off : off + sz].T
        m = dict(shared)
        m["sc_nat"] = sc20
        m["scT"] = np.ascontiguousarray(sc20.T.astype(BF16))
        m["ssT"] = np.ascontiguousarray(ssT.astype(BF16))
        m["bw"] = np.ascontiguousarray(bw_f[lo:hi].astype(BF16))
        m["bsm"] = np.ascontiguousarray(bsm_f[lo:hi].astype(BF16))
        m["qf"] = np.ascontiguousarray(qf_f[lo:hi].astype(BF16))
        in_maps.append(m)

    res = run_bass_kernel_spmd(nc, in_maps, core_ids=list(range(NCORES)))
    out = np.concatenate([res.results[c]["out"] for c in range(NCORES)], axis=0)
    return out.astype(np.float32)


# revision 11
# speedup vs baseline: 2.3153x; 1.1923x over previous
"""Trainium2 Bass kernel for nn_Classifier_22625887715977 (sparse_attention).

kernel(**inputs) takes FULL unsharded inputs (bs=32) and returns the full
[32, 75, 6] logits. Shards the batch over 8 NeuronCores (4 episodes per
core); weights replicated, converted to bf16 on host, streamed.

Math (per episode, exact reassociation of the reference — never materializes
the expanded per-(episode,way) base bank):
  s      = leaky(ss @ Wm1 + bm1) @ Wm2 + bm2
  avg    = mean_n [bw | bsm]
  gvis   = sigmoid(avg @ Wvis + bvis) + 1 ; gsem = sigmoid(avg @ Wsem + bsem) + 1
  q      = sc @ Wq + s @ Wqs
  scores = ((q @ Wk^T) * gvis) @ bw^T + ((q @ Wks^T) * gsem) @ bsm^T ; attn = softmax(scores/32)
  out    = ((attn @ bw) * gvis) @ Wv ; out2 = out @ Wfc + sc
  fake   = mean_w out2 ; protos = [sc; fake] ; logits = temp * cos(qf, protos)

v5 implementation notes:
 - All large streams (weights, banks, qf) are bf16 (host-converted); matmuls
   run bf16 x bf16 -> f32 PSUM. The residual / fake-proto path stays f32.
 - Wk, Wks, sc, ss are host-TRANSPOSED; projection outputs are computed
   directly in transposed form (qT, t1T, t2T, uT, outT) with tiny free sizes.
 - Gates are computed way-replicated ([B20, .] via an episode-selector lhsT)
   so each gate application is a single elementwise multiply.
 - DMA in global need-order: weights on sync; banks split across gpsimd and
   scalar queues so they land early; Wv / Wfc (tail deps) stream last.
 - Per-episode attention runs stage-parallel (scores x4 pipelined through
   softmax / attnT / uT) to hide cross-engine latency.
"""

import numpy as np
import ml_dtypes

BF16 = ml_dtypes.bfloat16

BS = 32
NCORES = 8
EPC = BS // NCORES          # 4 episodes per core
NW = 5
B20 = EPC * NW              # 20
FD = 1024
FDC = FD // 128             # 8
SEM = 300
SEMCH = [(0, 128), (128, 128), (256, 44)]
SEMP = 384                  # SEM padded to 3*128
NB = 512
NBC = NB // 128             # 4
NQ = 75
NPROTO = NW + 1             # 6
VINC = 11                   # ceil(1324/128)
VINP = VINC * 128           # 1408

_MODULE_CACHE = {}


def _build_module(temp: float):
    import concourse.mybir as mybir
    import concourse.tile as tile
    from concourse import bacc

    f32 = mybir.dt.float32
    f32r = mybir.dt.float32r
    bf = mybir.dt.bfloat16
    AF = mybir.ActivationFunctionType
    ALU = mybir.AluOpType
    AX = mybir.AxisListType

    nc = bacc.Bacc("TRN2", target_bir_lowering=False, debug=False)

    def di(name, shape, dt=f32):
        return nc.dram_tensor(name, shape, dt, kind="ExternalInput")

    sc_d = di("sc_nat", [B20, FD])
    fifths_d = di("fifths", [B20, EPC])
    bm1_d = di("bm1c", [128, 3])
    bm2_d = di("bm2c", [128, 3])
    scT_d = di("scT", [FD, B20], bf)
    ssT_d = di("ssT", [3, 128, B20], bf)
    esel_d = di("esel", [EPC, 128, B20], bf)
    ones_d = di("ones20", [1, B20], bf)
    bvis_d = di("bvis_row", [1, FD], bf)
    bsem_d = di("bsem_row", [1, SEM], bf)
    ident_d = di("ident_bf", [128, 128], bf)
    wm1_d = di("Wm1_pad", [SEMP, SEM], bf)
    wm2_d = di("Wm2_pad", [SEMP, SEM], bf)
    wq_d = di("Wq", [FD, FD], bf)
    wqs_d = di("Wqs_pad", [SEMP, FD], bf)
    wkT_d = di("WkT", [FD, FD], bf)
    wksT_d = di("WksT", [FD, SEM], bf)
    wv_d = di("Wv", [FD, FD], bf)
    wfc_d = di("Wfc", [FD, FD], bf)
    wvis_d = di("Wvis_pad", [VINP, FD], bf)
    wsem_d = di("Wsem_pad", [VINP, SEM], bf)
    bw_d = di("bw", [EPC, NB, FD], bf)
    bsm_d = di("bsm", [EPC, NB, SEM], bf)
    qf_d = di("qf", [EPC, NQ, FD], bf)
    out_d = nc.dram_tensor("out", [EPC, NQ, NPROTO], f32, kind="ExternalOutput")

    from contextlib import ExitStack
    with tile.TileContext(nc) as tc, ExitStack() as _ctx:
        def _pool(**kw):
            return _ctx.enter_context(tc.tile_pool(**kw))

        cpool = _pool(name="const", bufs=1)
        wres = _pool(name="wres", bufs=1)
        wbig = _pool(name="wbig", bufs=2)       # Wq,WkT then Wv,Wfc reuse
        wvp = _pool(name="wvis", bufs=1)
        bpool = _pool(name="banks", bufs=1)     # per-e tags
        tpool = _pool(name="bankT", bufs=1)     # per-e tags
        apool = _pool(name="acts", bufs=1)
        spool = _pool(name="small", bufs=1)
        ep2 = _pool(name="ep2", bufs=2)         # qf-normalize pipeline
        ep4 = _pool(name="ep4", bufs=4)         # attention stage pipeline
        qpool = _pool(name="qn", bufs=1)
        pt = _pool(name="pt", bufs=2, space="PSUM")
        pacc = _pool(name="pacc", bufs=2, space="PSUM")
        pproj = _pool(name="pproj", bufs=2, space="PSUM")
        pscore = _pool(name="pscore", bufs=2, space="PSUM")

        # ---------------- DMA issue ----------------
        # scalar queue: smalls, then odd-episode banks, then qf (cycling)
        ident = cpool.tile([128, 128], bf, tag="ident")
        nc.scalar.dma_start(ident[:], ident_d.ap())
        scT = cpool.tile([128, FDC, B20], bf, tag="scT")
        nc.scalar.dma_start(scT[:], scT_d.ap().rearrange("(c p) b -> p c b", p=128))
        ssT = cpool.tile([128, 3, B20], bf, tag="ssT")
        nc.scalar.dma_start(ssT[:], ssT_d.ap().rearrange("c p b -> p c b"))
        sc_nat = cpool.tile([B20, FD], f32, tag="sc_nat")
        nc.scalar.dma_start(sc_nat[:], sc_d.ap())
        esel = cpool.tile([128, EPC, B20], bf, tag="esel")
        nc.scalar.dma_start(esel[:], esel_d.ap().rearrange("e p b -> p e b"))
        fifths = cpool.tile([B20, EPC], f32, tag="fifths")
        nc.scalar.dma_start(fifths[:], fifths_d.ap())
        ones20 = cpool.tile([1, B20], bf, tag="ones20")
        nc.scalar.dma_start(ones20[:], ones_d.ap())
        bm1c = cpool.tile([128, 3], f32, tag="bm1c")
        nc.scalar.dma_start(bm1c[:], bm1_d.ap())
        bm2c = cpool.tile([128, 3], f32, tag="bm2c")
        nc.scalar.dma_start(bm2c[:], bm2_d.ap())
        bvis_row = cpool.tile([1, FD], bf, tag="bvis")
        nc.scalar.dma_start(bvis_row[:], bvis_d.ap())
        bsem_row = cpool.tile([1, SEM], bf, tag="bsem")
        nc.scalar.dma_start(bsem_row[:], bsem_d.ap())
        wm1 = wres.tile([128, 3, SEM], bf, tag="wm1")
        nc.scalar.dma_start(wm1[:], wm1_d.ap().rearrange("(c p) d -> p c d", p=128))
        wm2 = wres.tile([128, 3, SEM], bf, tag="wm2")
        nc.scalar.dma_start(wm2[:], wm2_d.ap().rearrange("(c p) d -> p c d", p=128))

        bw_nat, bsm_nat = [None] * EPC, [None] * EPC
        for e in range(EPC):
            bwt = bpool.tile([128, NBC, FD], bf, tag=f"bw{e}")
            nc.gpsimd.dma_start(bwt[:], bw_d.ap()[e].rearrange("(c p) d -> p c d", p=128))
            bw_nat[e] = bwt
            bst = bpool.tile([128, NBC, SEM], bf, tag=f"bsm{e}")
            nc.gpsimd.dma_start(bst[:], bsm_d.ap()[e].rearrange("(c p) d -> p c d", p=128))
            bsm_nat[e] = bst

        # sync queue: big weights in need-order
        wq = wbig.tile([128, FDC, FD], bf, tag="wbig")
        nc.sync.dma_start(wq[:], wq_d.ap().rearrange("(c p) d -> p c d", p=128))
        wkT = wbig.tile([128, FDC, FD], bf, tag="wbig")
        nc.sync.dma_start(wkT[:], wkT_d.ap().rearrange("(c p) d -> p c d", p=128))
        wqs = wres.tile([128, 3, FD], bf, tag="wqs")
        nc.scalar.dma_start(wqs[:], wqs_d.ap().rearrange("(c p) d -> p c d", p=128))
        wksT = wres.tile([128, FDC, SEM], bf, tag="wksT")
        nc.scalar.dma_start(wksT[:], wksT_d.ap().rearrange("(c p) d -> p c d", p=128))
        wvis = wvp.tile([128, VINC, FD], bf, tag="wvis")
        nc.sync.dma_start(wvis[:], wvis_d.ap().rearrange("(c p) d -> p c d", p=128))
        wsem = wvp.tile([128, VINC, SEM], bf, tag="wsem")
        nc.scalar.dma_start(wsem[:], wsem_d.ap().rearrange("(c p) d -> p c d", p=128))
        wv = wbig.tile([128, FDC, FD], bf, tag="wbig")
        nc.sync.dma_start(wv[:], wv_d.ap().rearrange("(c p) d -> p c d", p=128))
        wfc = wbig.tile([128, FDC, FD], bf, tag="wbig")
        for h in range(2):
            nc.sync.dma_start(
                wfc[:, :, h * 512 : (h + 1) * 512],
                wfc_d.ap()[:, h * 512 : (h + 1) * 512]
                .rearrange("(c p) d -> p c d", p=128))

        # memset partial-chunk tiles that are read at full 128 partitions
        h1T = apool.tile([128, 3, B20], bf, tag="h1T")
        sT = apool.tile([128, 3, B20], bf, tag="sT")
        avgsT = apool.tile([128, 3, B20], bf, tag="avgsT")
        nc.vector.memset(h1T[:], 0.0)
        nc.vector.memset(sT[:], 0.0)
        nc.vector.memset(avgsT[:], 0.0)

        _ei = [0]

        def copy_rr(dst, src):
            _ei[0] += 1
            if _ei[0] % 2:
                nc.vector.tensor_copy(dst, src)
            else:
                nc.scalar.copy(dst, src)

        # ---------------- sMLP ----------------
        for mc, (moff, msz) in enumerate(SEMCH):
            ph = pproj.tile([128, B20], f32, tag="ps_proj")
            for kc, (koff, ksz) in enumerate(SEMCH):
                nc.tensor.matmul(ph[0:msz, :], wm1[0:ksz, kc, moff : moff + msz],
                                 ssT[0:ksz, kc, :], start=(kc == 0), stop=(kc == 2))
            lk = spool.tile([128, B20], f32, tag="mlp_lk")
            nc.vector.tensor_scalar(lk[0:msz, :], ph[0:msz, :], bm1c[0:msz, mc : mc + 1],
                                    0.1, op0=ALU.add, op1=ALU.mult)
            nc.vector.tensor_scalar(h1T[0:msz, mc, :], ph[0:msz, :],
                                    bm1c[0:msz, mc : mc + 1], None, op0=ALU.add)
            nc.vector.tensor_tensor(h1T[0:msz, mc, :], h1T[0:msz, mc, :], lk[0:msz, :],
                                    op=ALU.max)
        for mc, (moff, msz) in enumerate(SEMCH):
            ph = pproj.tile([128, B20], f32, tag="ps_proj")
            for kc, (koff, ksz) in enumerate(SEMCH):
                nc.tensor.matmul(ph[0:msz, :], wm2[0:ksz, kc, moff : moff + msz],
                                 h1T[0:ksz, kc, :], start=(kc == 0), stop=(kc == 2))
            nc.vector.tensor_scalar(sT[0:msz, mc, :], ph[0:msz, :],
                                    bm2c[0:msz, mc : mc + 1], None, op0=ALU.add)

        # ---------------- pn_sc = sc/||sc|| (early) + pnT sc columns ----------------
        ssq_sc = spool.tile([B20, 1], f32, tag="sc_ssq")
        sq_sc = spool.tile([NQ, FD], bf, tag="scratch4k")
        nc.scalar.activation(sq_sc[0:B20, :], sc_nat[:], AF.Square, accum_out=ssq_sc[:])
        r_sc = spool.tile([B20, 1], f32, tag="sc_r")
        nc.vector.reciprocal(r_sc[:], ssq_sc[:])
        inv_sc = spool.tile([B20, 1], f32, tag="sc_inv")
        nc.scalar.activation(inv_sc[:], r_sc[:], AF.Sqrt)
        pn_sc = spool.tile([B20, FD], bf, tag="pn_sc")
        nc.vector.tensor_scalar(pn_sc[:], sc_nat[:], inv_sc[:], None, op0=ALU.mult)
        pnT = apool.tile([128, FDC, EPC * NPROTO], bf, tag="pnT")
        for g in range(2):
            tfull = pt.tile([128, 1024], bf, tag="tr")
            t = tfull[:, 0 : 4 * B20]
            for i in range(4):
                dc = g * 4 + i
                nc.tensor.transpose(t[:, i * B20 : (i + 1) * B20],
                                    pn_sc[:, dc * 128 : (dc + 1) * 128], ident[0:B20, 0:B20])
            for i in range(4):
                dc = g * 4 + i
                dst = pnT[:, dc, :].rearrange("p (e s) -> p e s", s=NPROTO)[:, :, 0:NW]
                src = t[:, i * B20 : (i + 1) * B20].rearrange("p (e w) -> p e w", w=NW)
                nc.vector.tensor_copy(dst, src)

        # ---------------- qT = (sc@Wq + s@Wqs)^T directly ----------------
        qT_ps = pproj.tile([128, FDC, B20], f32, tag="ps_proj")
        for m in range(FDC):
            for kc in range(FDC):
                nc.tensor.matmul(qT_ps[:, m, :], wq[:, kc, m * 128 : (m + 1) * 128],
                                 scT[:, kc, :], start=(kc == 0), stop=False)
            for c in range(3):
                nc.tensor.matmul(qT_ps[:, m, :], wqs[:, c, m * 128 : (m + 1) * 128],
                                 sT[:, c, :], start=False, stop=(c == 2))
        qT = apool.tile([128, FDC, B20], bf, tag="qT")
        nc.vector.tensor_copy(qT[:], qT_ps[:])

        # ---------------- per-episode banks: avg chunks + transposes ----------------
        avg_bank = pacc.tile([128, 512], f32, tag="ps_acc")
        avgv_ps = [avg_bank[0:B20, :], avg_bank[32 : 32 + B20, :]]
        avgs_ps = avg_bank[64 : 64 + B20, 0:SEM]
        bwT_l, bsmT_l = [], []
        t1_done = False
        t1c = apool.tile([128, FDC, B20], bf, tag="t1c")
        t2c = apool.tile([128, 3, B20], bf, tag="t2c")
        for e in range(EPC):
            for c in range(NBC):
                for h in range(2):
                    nc.tensor.matmul(avgv_ps[h], esel[:, e, :],
                                     bw_nat[e][:, c, h * 512 : (h + 1) * 512],
                                     start=(e == 0 and c == 0),
                                     stop=(e == EPC - 1 and c == NBC - 1))
            bwT = tpool.tile([128, FDC, NB], bf, tag=f"bwT{e}")
            for g in range(4):
                t = pt.tile([128, 1024], bf, tag="tr")
                for i in range(2):
                    dc = g * 2 + i
                    for c in range(NBC):
                        nc.tensor.transpose(
                            t[:, i * 512 + c * 128 : i * 512 + (c + 1) * 128],
                            bw_nat[e][:, c, dc * 128 : (dc + 1) * 128],
                            ident[:])
                copy_rr(bwT[:, g * 2 : g * 2 + 2, :], t[:])
            bwT_l.append(bwT)
            for c in range(NBC):
                nc.tensor.matmul(avgs_ps, esel[:, e, :], bsm_nat[e][:, c, :],
                                 start=(e == 0 and c == 0),
                                 stop=(e == EPC - 1 and c == NBC - 1))
            bsmT = tpool.tile([128, 3, NB], bf, tag=f"bsmT{e}")
            t2p = pt.tile([128, 1024], bf, tag="tr")
            for sci, (soff, ssz) in enumerate(SEMCH[:2]):
                for c in range(NBC):
                    nc.tensor.transpose(
                        t2p[:, sci * 512 + c * 128 : sci * 512 + (c + 1) * 128],
                        bsm_nat[e][:, c, soff : soff + ssz], ident[:])
            copy_rr(bsmT[:, 0:2, :], t2p[:])
            t3p = pt.tile([128, 1024], bf, tag="tr")
            soff, ssz = SEMCH[2]
            for c in range(NBC):
                nc.tensor.transpose(t3p[0:ssz, c * 128 : (c + 1) * 128],
                                    bsm_nat[e][:, c, soff : soff + ssz],
                                    ident[:])
            copy_rr(bsmT[0:ssz, 2, :], t3p[0:ssz, 0:512])
            bsmT_l.append(bsmT)

            if not t1_done:
                # t1T/t2T after first bank (WkT arrives around now)
                t1_done = True
                t1_ps = pproj.tile([128, FDC, B20], f32, tag="ps_proj")
                for m in range(FDC):
                    for kc in range(FDC):
                        nc.tensor.matmul(t1_ps[:, m, :],
                                         wkT[:, kc, m * 128 : (m + 1) * 128],
                                         qT[:, kc, :], start=(kc == 0),
                                         stop=(kc == FDC - 1))
                nc.vector.tensor_copy(t1c[:], t1_ps[:])
                t2_ps = pproj.tile([128, 3, B20], f32, tag="ps_proj")
                for mc, (moff, msz) in enumerate(SEMCH):
                    for kc in range(FDC):
                        nc.tensor.matmul(t2_ps[0:msz, mc, :],
                                         wksT[:, kc, moff : moff + msz],
                                         qT[:, kc, :], start=(kc == 0),
                                         stop=(kc == FDC - 1))
                nc.vector.tensor_copy(t2c[:], t2_ps[:])

        # ---------------- avg transposes -> way-replicated avgT ----------------
        avgv_nat = spool.tile([B20, FD], bf, tag="avgv_nat")
        for h in range(2):
            nc.vector.tensor_copy(avgv_nat[:, h * 512 : (h + 1) * 512], avgv_ps[h])
        avgs_nat = spool.tile([B20, SEM], bf, tag="avgs_nat")
        nc.vector.tensor_copy(avgs_nat[:], avgs_ps)
        avgvT = apool.tile([128, FDC, B20], bf, tag="avgvT")
        for g in range(2):
            tfull = pt.tile([128, 1024], bf, tag="tr")
            t = tfull[:, 0 : 4 * B20]
            for i in range(4):
                dc = g * 4 + i
                nc.tensor.transpose(t[:, i * B20 : (i + 1) * B20],
                                    avgv_nat[:, dc * 128 : (dc + 1) * 128],
                                    ident[0:B20, 0:B20])
            copy_rr(avgvT[:, g * 4 : (g + 1) * 4, :], t[:])
        tsp_f = pt.tile([128, 1024], bf, tag="tr")
        tsp = tsp_f[:, 0 : 3 * B20]
        for sci, (soff, ssz) in enumerate(SEMCH):
            nc.tensor.transpose(tsp[0:ssz, sci * B20 : (sci + 1) * B20],
                                avgs_nat[:, soff : soff + ssz], ident[0:B20, 0:B20])
        nc.vector.tensor_copy(avgsT[0:128, 0, :], tsp[0:128, 0:B20])
        nc.vector.tensor_copy(avgsT[0:128, 1, :], tsp[0:128, B20 : 2 * B20])
        soff, ssz = SEMCH[2]
        nc.vector.tensor_copy(avgsT[0:ssz, 2, :], tsp[0:ssz, 2 * B20 : 3 * B20])

        # ---------------- gates ----------------
        g_bank = pacc.tile([128, 512], f32, tag="ps_acc")
        gpv_ps = [g_bank[0:B20, :], g_bank[32 : 32 + B20, :]]
        gps_ps = g_bank[64 : 64 + B20, 0:SEM]
        for h in range(2):
            for kc in range(FDC):
                nc.tensor.matmul(gpv_ps[h], avgvT[:, kc, :],
                                 wvis[:, kc, h * 512 : (h + 1) * 512],
                                 start=(kc == 0), stop=False)
            for c in range(3):
                nc.tensor.matmul(gpv_ps[h], avgsT[:, c, :],
                                 wvis[:, FDC + c, h * 512 : (h + 1) * 512],
                                 start=False, stop=False)
            nc.tensor.matmul(gpv_ps[h], ones20[:], bvis_row[:, h * 512 : (h + 1) * 512],
                             start=False, stop=True)
        for kc in range(FDC):
            nc.tensor.matmul(gps_ps, avgvT[:, kc, :], wsem[:, kc, :],
                             start=(kc == 0), stop=False)
        for c in range(3):
            nc.tensor.matmul(gps_ps, avgsT[:, c, :], wsem[:, FDC + c, :],
                             start=False, stop=False)
        nc.tensor.matmul(gps_ps, ones20[:], bsem_row[:], start=False, stop=True)

        gv_nat = spool.tile([B20, FD], bf, tag="avgv_nat")
        for h in range(2):
            nc.scalar.activation(gv_nat[:, h * 512 : (h + 1) * 512], gpv_ps[h],
                                 AF.Sigmoid)
        gs_nat = spool.tile([B20, SEM], bf, tag="avgs_nat")
        nc.scalar.activation(gs_nat[:], gps_ps, AF.Sigmoid)

        gvis5T = apool.tile([128, FDC, B20], bf, tag="gvis5T")
        for g in range(2):
            tfull = pt.tile([128, 1024], bf, tag="tr")
            t = tfull[:, 0 : 4 * B20]
            for i in range(4):
                dc = g * 4 + i
                nc.tensor.transpose(t[:, i * B20 : (i + 1) * B20],
                                    gv_nat[:, dc * 128 : (dc + 1) * 128],
                                    ident[0:B20, 0:B20])
            nc.vector.tensor_scalar_add(gvis5T[:, g * 4 : (g + 1) * 4, :], t[:], 1.0)
        gsem5T = apool.tile([128, 3, B20], bf, tag="gsem5T")
        tg_f = pt.tile([128, 1024], bf, tag="tr")
        tg = tg_f[:, 0 : 3 * B20]
        for sci, (soff, ssz) in enumerate(SEMCH):
            nc.tensor.transpose(tg[0:ssz, sci * B20 : (sci + 1) * B20],
                                gs_nat[:, soff : soff + ssz], ident[0:B20, 0:B20])
        nc.vector.tensor_scalar_add(gsem5T[0:128, 0:2, :],
                                    tg[0:128, 0 : 2 * B20].rearrange("p (c b) -> p c b", c=2),
                                    1.0)
        soff, ssz = SEMCH[2]
        nc.vector.tensor_scalar_add(gsem5T[0:ssz, 2, :], tg[0:ssz, 2 * B20 : 3 * B20], 1.0)

        t1gT = apool.tile([128, FDC, B20], bf, tag="t1gT")
        nc.vector.tensor_tensor(t1gT[:], t1c[:], gvis5T[:], op=ALU.mult)
        t2gT = apool.tile([128, 3, B20], bf, tag="t2gT")
        nc.vector.tensor_tensor(t2gT[:], t2c[:], gsem5T[:], op=ALU.mult)

        # ---------------- attention: stage-parallel over episodes ----------------
        def scores_mm(e):
            sc_ps = pscore.tile([NW, NB], f32, tag="ps_sc")
            for dc in range(FDC):
                nc.tensor.matmul(sc_ps[:], t1gT[:, dc, e * NW : (e + 1) * NW],
                                 bwT_l[e][:, dc, :], start=(dc == 0), stop=False)
            for sci, (soff, ssz) in enumerate(SEMCH):
                nc.tensor.matmul(sc_ps[:], t2gT[0:ssz, sci, e * NW : (e + 1) * NW],
                                 bsmT_l[e][0:ssz, sci, :], start=False, stop=(sci == 2))
            return sc_ps

        def softmax(e, sc_ps):
            mx = ep4.tile([NW, 1], f32, tag="mx")
            nc.vector.reduce_max(mx[:], sc_ps[:], axis=AX.X)
            mxn = ep4.tile([NW, 1], f32, tag="mxn")
            nc.vector.tensor_scalar(mxn[:], mx[:], -1.0 / 32.0, None, op0=ALU.mult)
            attn = ep4.tile([NW, NB], bf, tag="attn")
            sm = ep4.tile([NW, 1], f32, tag="sm")
            nc.scalar.activation(attn[:], sc_ps[:], AF.Exp, bias=mxn[:], scale=1.0 / 32.0,
                                 accum_out=sm[:])
            rs = ep4.tile([NW, 1], f32, tag="rs")
            nc.vector.reciprocal(rs[:], sm[:])
            nc.vector.tensor_scalar(attn[:], attn[:], rs[:], None, op0=ALU.mult)
            return attn

        def attnT_mm(e, attn):
            attnT = ep4.tile([128, NBC, NW], bf, tag="attnT")
            ta_f = pt.tile([128, 1024], bf, tag="tr")
            ta = ta_f[:, 0 : NBC * NW]
            for c in range(NBC):
                nc.tensor.transpose(ta[:, c * NW : (c + 1) * NW],
                                    attn[:, c * 128 : (c + 1) * 128], ident[0:NW, 0:NW])
            nc.vector.tensor_copy(attnT[:], ta[:])
            return attnT

        ugT = apool.tile([128, FDC, B20], bf, tag="ugT")

        def uT_mm(e, attnT):
            uT_ps = pproj.tile([128, FDC, NW], f32, tag="ps_proj")
            for dc in range(FDC):
                for c in range(NBC):
                    nc.tensor.matmul(uT_ps[:, dc, :],
                                     bw_nat[e][:, c, dc * 128 : (dc + 1) * 128],
                                     attnT[:, c, :], start=(c == 0), stop=(c == NBC - 1))
            nc.vector.tensor_tensor(ugT[:, :, e * NW : (e + 1) * NW], uT_ps[:],
                                    gvis5T[:, :, e * NW : (e + 1) * NW], op=ALU.mult)

        sc_ps_l = [None] * EPC
        attn_l = [None] * EPC
        attnT_list = [None] * EPC
        sc_ps_l[0] = scores_mm(0)
        sc_ps_l[1] = scores_mm(1)
        attn_l[0] = softmax(0, sc_ps_l[0])
        attnT_list[0] = attnT_mm(0, attn_l[0])
        sc_ps_l[2] = scores_mm(2)
        attn_l[1] = softmax(1, sc_ps_l[1])
        uT_mm(0, attnT_list[0])
        attnT_list[1] = attnT_mm(1, attn_l[1])
        sc_ps_l[3] = scores_mm(3)
        attn_l[2] = softmax(2, sc_ps_l[2])
        uT_mm(1, attnT_list[1])
        attnT_list[2] = attnT_mm(2, attn_l[2])
        attn_l[3] = softmax(3, sc_ps_l[3])
        uT_mm(2, attnT_list[2])
        attnT_list[3] = attnT_mm(3, attn_l[3])
        uT_mm(3, attnT_list[3])

        # ---------------- qf normalize + qnT (banks done; qf arrives now) -------
        qnT_l = []
        for e in range(EPC):
            qf_e = ep2.tile([NQ, FD], bf, tag="qf")
            nc.scalar.dma_start(qf_e[:], qf_d.ap()[e])
            ssq = ep2.tile([NQ, 1], f32, tag="q_ssq")
            sq = spool.tile([NQ, FD], bf, tag="scratch4k")
            nc.scalar.activation(sq[:], qf_e[:], AF.Square, accum_out=ssq[:])
            rq = ep2.tile([NQ, 1], f32, tag="q_rq")
            nc.vector.reciprocal(rq[:], ssq[:])
            s10 = ep2.tile([NQ, 1], f32, tag="q_s10")
            nc.scalar.activation(s10[:], rq[:], AF.Sqrt, scale=float(temp) * float(temp))
            qn = ep2.tile([NQ, FD], bf, tag="q_qn")
            nc.vector.tensor_scalar(qn[:], qf_e[:], s10[:], None, op0=ALU.mult)
            qnT = qpool.tile([128, FDC, NQ], bf, tag=f"qnT{e}")
            for g in range(2):
                tfull = pt.tile([128, 1024], bf, tag="tr")
                t = tfull[:, 0 : 4 * NQ]
                for i in range(4):
                    dc = g * 4 + i
                    nc.tensor.transpose(t[:, i * NQ : (i + 1) * NQ],
                                        qn[:, dc * 128 : (dc + 1) * 128], ident[0:NQ, 0:NQ])
                copy_rr(qnT[:, g * 4 : (g + 1) * 4, :], t[:])
            qnT_l.append(qnT)

        # ---------------- outT = ((u*g) @ Wv)^T ----------------
        outT_ps = pproj.tile([128, FDC, B20], f32, tag="ps_proj")
        for m in range(FDC):
            for kc in range(FDC):
                nc.tensor.matmul(outT_ps[:, m, :], wv[:, kc, m * 128 : (m + 1) * 128],
                                 ugT[:, kc, :], start=(kc == 0), stop=(kc == FDC - 1))
        outT = apool.tile([128, FDC, B20], bf, tag="outT")
        nc.vector.tensor_copy(outT[:], outT_ps[:])

        # ---------------- out2 = out@Wfc + sc ; fake (per half) ----------------
        out2 = apool.tile([B20, FD], f32, tag="out2")
        fk = spool.tile([EPC, FD], f32, tag="fk")
        ssf_h0 = spool.tile([EPC, 1], f32, tag="fk_ssq0")
        ssf_h1 = spool.tile([EPC, 1], f32, tag="fk_ssq1")
        ssf_h = [ssf_h0, ssf_h1]
        sqf = spool.tile([NQ, FD], bf, tag="scratch4k")
        o_bank = pacc.tile([128, 512], f32, tag="ps_acc")
        f_bank = pacc.tile([128, 512], f32, tag="ps_acc")
        for h in range(2):
            o2_ps = o_bank[h * 32 : h * 32 + B20, :]
            for kc in range(FDC):
                nc.tensor.matmul(o2_ps, outT[:, kc, :],
                                 wfc[:, kc, h * 512 : (h + 1) * 512],
                                 start=(kc == 0), stop=(kc == FDC - 1))
            nc.vector.tensor_tensor(out2[:, h * 512 : (h + 1) * 512], o2_ps,
                                    sc_nat[:, h * 512 : (h + 1) * 512], op=ALU.add)
            fk_ps = f_bank[h * 32 : h * 32 + EPC, :]
            nc.tensor.matmul(fk_ps, fifths[:].bitcast(f32r),
                             out2[:, h * 512 : (h + 1) * 512].bitcast(f32r),
                             start=True, stop=True)
            nc.vector.tensor_copy(fk[:, h * 512 : (h + 1) * 512], fk_ps)
            nc.scalar.activation(sqf[h * EPC : (h + 1) * EPC, 0:512],
                                 fk[:, h * 512 : (h + 1) * 512], AF.Square,
                                 accum_out=ssf_h[h][:])
        ssf = spool.tile([EPC, 1], f32, tag="fk_ssq")
        nc.vector.tensor_tensor(ssf[:], ssf_h0[:], ssf_h1[:], op=ALU.add)
        rf = spool.tile([EPC, 1], f32, tag="fk_r")
        nc.vector.reciprocal(rf[:], ssf[:])
        inv_f = spool.tile([EPC, 1], f32, tag="fk_inv")
        nc.scalar.activation(inv_f[:], rf[:], AF.Sqrt)
        pn_fk = spool.tile([EPC, FD], bf, tag="pn_fk")
        nc.vector.tensor_scalar(pn_fk[:], fk[:], inv_f[:], None, op0=ALU.mult)
        tf_f = pt.tile([128, 1024], bf, tag="tr")
        tf = tf_f[:, 0 : FDC * EPC]
        for dc in range(FDC):
            nc.tensor.transpose(tf[:, dc * EPC : (dc + 1) * EPC],
                                pn_fk[:, dc * 128 : (dc + 1) * 128], ident[0:EPC, 0:EPC])
        dst = pnT[:].rearrange("p c (e s) -> p c e s", s=NPROTO)[:, :, :, NW]
        nc.vector.tensor_copy(dst, tf[:].rearrange("p (c e) -> p c e", e=EPC))

        # ---------------- logits ----------------
        lg_ps = pproj.tile([NQ, EPC * NPROTO], f32, tag="ps_proj")
        for e in range(EPC):
            for dc in range(FDC):
                nc.tensor.matmul(lg_ps[:, e * NPROTO : (e + 1) * NPROTO],
                                 qnT_l[e][:, dc, :],
                                 pnT[:, dc, e * NPROTO : (e + 1) * NPROTO],
                                 start=(dc == 0), stop=(dc == FDC - 1))
        lg = spool.tile([NQ, EPC * NPROTO], f32, tag="lg")
        nc.vector.tensor_copy(lg[:], lg_ps[:])
        for e in range(EPC):
            nc.scalar.dma_start(out_d.ap()[e], lg[:, e * NPROTO : (e + 1) * NPROTO])

    nc.finalize()
    return nc


def _prep_shared(inputs):
    """bf16-convert / transpose / pad the replicated weights (host side)."""
    def b(a):
        return np.ascontiguousarray(np.asarray(a).astype(BF16))

    def padr(a, n):
        p = np.zeros((n - a.shape[0],) + a.shape[1:], a.dtype)
        return np.ascontiguousarray(np.concatenate([a, p], axis=0))

    f32 = np.float32
    bm1c = np.zeros((128, 3), f32)
    bm2c = np.zeros((128, 3), f32)
    bm1 = np.asarray(inputs["bm1"], f32).reshape(-1)
    bm2 = np.asarray(inputs["bm2"], f32).reshape(-1)
    for c, (off, sz) in enumerate(SEMCH):
        bm1c[0:sz, c] = bm1[off : off + sz]
        bm2c[0:sz, c] = bm2[off : off + sz]

    esel = np.zeros((EPC, 128, B20), f32)
    for e in range(EPC):
        esel[e, :, e * NW : (e + 1) * NW] = 1.0 / NB
    fifths = np.zeros((B20, EPC), f32)
    for e in range(EPC):
        fifths[e * NW : (e + 1) * NW, e] = 1.0 / NW

    return {
        "Wm1_pad": b(padr(np.asarray(inputs["Wm1"], f32), SEMP)),
        "Wm2_pad": b(padr(np.asarray(inputs["Wm2"], f32), SEMP)),
        "Wq": b(inputs["Wq"]),
        "Wqs_pad": b(padr(np.asarray(inputs["Wqs"], f32), SEMP)),
        "WkT": b(np.asarray(inputs["Wk"], f32).T),
        "WksT": b(np.asarray(inputs["Wks"], f32).T),
        "Wv": b(inputs["Wv"]),
        "Wfc": b(inputs["Wfc"]),
        "Wvis_pad": b(padr(np.asarray(inputs["Wvis"], f32), VINP)),
        "Wsem_pad": b(padr(np.asarray(inputs["Wsem"], f32), VINP)),
        "bm1c": bm1c,
        "bm2c": bm2c,
        "bvis_row": b(np.asarray(inputs["bvis"], f32).reshape(1, FD)),
        "bsem_row": b(np.asarray(inputs["bsem"], f32).reshape(1, SEM)),
        "esel": b(esel),
        "fifths": fifths,
        "ones20": b(np.ones((1, B20), f32)),
        "ident_bf": b(np.eye(128, dtype=f32)),
    }


def kernel(**inputs):
    from concourse.bass_utils import run_bass_kernel_spmd

    temp = float(np.asarray(inputs["temp"]))
    key = ("v5", temp)
    if key not in _MODULE_CACHE:
        _MODULE_CACHE[key] = _build_module(temp)
    nc = _MODULE_CACHE[key]

    shared = _prep_shared(inputs)
    sc_f = np.asarray(inputs["support_center"], np.float32)
    ss_f = np.asarray(inputs["support_seman"], np.float32)
    bw_f = np.asarray(inputs["base_weights"])
    bsm_f = np.asarray(inputs["base_seman"])
    qf_f = np.asarray(inputs["query_feature"])

    in_maps = []
    for cid in range(NCORES):
        lo, hi = cid * EPC, (cid + 1) * EPC
        sc20 = np.ascontiguousarray(sc_f[lo:hi].reshape(B20, FD))
        ss20 = ss_f[lo:hi].reshape(B20, SEM)
        ssT = np.zeros((3, 128, B20), np.float32)
        for c, (off, sz) in enumerate(SEMCH):
            ssT[c, 0:sz, :] = ss20[:, off : off + sz].T
        m = dict(shared)
        m["sc_nat"] = sc20
        m["scT"] = np.ascontiguousarray(sc20.T.astype(BF16))
        m["ssT"] = np.ascontiguousarray(ssT.astype(BF16))
        m["bw"] = np.ascontiguousarray(bw_f[lo:hi].astype(BF16))
        m["bsm"] = np.ascontiguousarray(bsm_f[lo:hi].astype(BF16))
        m["qf"] = np.ascontiguousarray(qf_f[lo:hi].astype(BF16))
        in_maps.append(m)

    res = run_bass_kernel_spmd(nc, in_maps, core_ids=list(range(NCORES)))
    out = np.concatenate([res.results[c]["out"] for c in range(NCORES)], axis=0)
    return out.astype(np.float32)
